# revision 1
# baseline (speedup 1.0000x reference)
"""Trainium2 Bass kernel for a 2-layer GAT (nn_AGAEMD problem).

Sharding: layer-1 heads across 8 cores (core h owns head h, full N x N
attention for that head); layer-2 row-sharded (core c owns output rows
[c*512, (c+1)*512)).  Head outputs are combined with a bf16
ReduceScatter+AllGather of the partial out-projection (elu(O_h) @ Wo_h),
which also carries the layer-2 attention logits g_src/g_dst.

Math notes:
 - softmax(masked e) is computed unnormalized: P = exp(leaky(e)) * adj
   (exact since adj is 0/1 and values are small enough for exp in bf16),
   with the row sum Z obtained by a fused ones-column in the matmul lhsT.
 - exp(leaky(s)), s = fs_i + fd_j, is separable:
   exp(leaky(s)) = max(exp(fs_i)exp(fd_j), exp(.2 fs_i)exp(.2 fd_j)),
   which lets the work run as DVE tensor_scalar/tensor_tensor ops; a
   second pipeline computes it directly as Lrelu+Exp on the scalar
   engine.  Tiles are split across ACT/DVE/GPSIMD pipelines.
 - elu(x) = max(x,0) - 1 + exp(min(x,0)).
"""

import sys

if "/opt/trn_rl_repo" not in sys.path:
    sys.path.insert(0, "/opt/trn_rl_repo")

import numpy as np
import ml_dtypes

BF = ml_dtypes.bfloat16

# problem dims (hardcoded per spec)
N, F, H, D, C = 4096, 256, 8, 64, 64
CORES = 8
SLOPE = 0.2

# engine-split tunables (layer 1): of each 16 j-tiles, first ACT1 go to the
# scalar-engine pipeline, rest to the DVE pipeline; within DVE-pipe tiles the
# max-combine alternates DVE/GPSIMD; ACT-pipe mask-mults go to GPSIMD.
import os as _os

ACT1_PER16 = int(_os.environ.get("K_ACT1", "7"))
ACT2_PER16 = int(_os.environ.get("K_ACT2", "7"))
USE_GPS = int(_os.environ.get("K_GPS", "1"))
K_DEBUG = int(_os.environ.get("K_DEBUG", "0"))

_BASS_CACHE = {}


def _emit(nc, tc, n, shard):
    """Emit the SPMD per-core graph. n = graph size (4096 full), shard = n//8."""
    import concourse.bass as bass
    import concourse.mybir as mybir
    from concourse.masks import make_identity

    bf = mybir.dt.bfloat16
    f32 = mybir.dt.float32
    AF = mybir.ActivationFunctionType
    OP = mybir.AluOpType
    NT = n // 128          # number of 128-row tiles
    NH = n // 2            # split-layout free width
    RG = [list(range(CORES))]

    # ---- dram I/O ----
    xT_d = nc.dram_tensor("xT", [F, n], bf, kind="ExternalInput")
    adjT_d = nc.dram_tensor("adjT", [n, n], bf, kind="ExternalInput")
    adjs_d = nc.dram_tensor("adjs", [n, shard], bf, kind="ExternalInput")
    wh_d = nc.dram_tensor("wh", [F, D], bf, kind="ExternalInput")
    whT_d = nc.dram_tensor("whT", [D, F], bf, kind="ExternalInput")
    a2_d = nc.dram_tensor("a2", [D, 2], bf, kind="ExternalInput")
    wo_d = nc.dram_tensor("wo", [D, C], bf, kind="ExternalInput")
    woT_d = nc.dram_tensor("woT", [C, D], bf, kind="ExternalInput")
    ao2_d = nc.dram_tensor("ao2", [C, 2], bf, kind="ExternalInput")
    out_d = nc.dram_tensor("out", [shard, C], f32, kind="ExternalOutput")

    # collective bounce buffers
    rs_in = nc.dram_tensor("rs_in", [n, C + 2], bf)
    rs_out = nc.dram_tensor("rs_out", [shard, C + 2], bf)
    ag_out = nc.dram_tensor("ag_out", [n, C + 2], bf, addr_space="Shared")

    from contextlib import ExitStack

    es = ExitStack()
    pers = es.enter_context(tc.tile_pool(name="pers", bufs=1))
    ppool = es.enter_context(tc.tile_pool(name="psum", bufs=1, space="PSUM"))
    pbig = ppool.tile([128, 4096], f32, name="pbig")

    # ---- prologue: weights ----
    xtp = tc.tile_pool(name="xtp", bufs=1)
    xtpool = xtp.__enter__()
    xt = []
    for k in range(2):
        t = xtpool.tile([128, n], bf, name=f"xt{k}")
        nc.sync.dma_start(t[:], xT_d[k * 128:(k + 1) * 128, :])
        xt.append(t)
    wf = []
    for k in range(2):
        t = pers.tile([128, D + 2], bf, name=f"wf{k}")
        nc.sync.dma_start(t[:, 0:D], wh_d[k * 128:(k + 1) * 128, :])
        wf.append(t)
    whTt = pers.tile([128, F], bf, name="whTt")
    nc.gpsimd.memset(whTt[:], 0.0)
    nc.sync.dma_start(whTt[0:D, :], whT_d[:])
    a2t = pers.tile([128, 2], bf, name="a2t")
    nc.gpsimd.memset(a2t[:], 0.0)
    nc.sync.dma_start(a2t[0:D, :], a2_d[:])

    # waug = W_h @ a2 : [F, 2] (two 128-row tiles)
    for k in range(2):
        pw = pbig[0:128, k * 512:k * 512 + 2]
        nc.tensor.matmul(pw, lhsT=whTt[:, k * 128:(k + 1) * 128], rhs=a2t[:],
                         start=True, stop=True)
        nc.vector.tensor_copy(wf[k][:, D:D + 2], pw)

    # fsrc row via matmul: fsrc = x @ wsrc -> psum rows, split on partitions 0/1
    n_cc = n // 512
    cpg = max(1, NH // 512)  # 512-chunks per half
    for cc in range(n_cc):
        part = (cc // cpg) * 32  # matmul out base partition must be 0/32/64
        foff = 2048 + 512 * (cc % cpg)
        pr = pbig[part:part + 1, foff:foff + 512]
        for k in range(2):
            nc.tensor.matmul(pr, lhsT=wf[k][:, D:D + 1],
                             rhs=xt[k][:, cc * 512:(cc + 1) * 512],
                             start=(k == 0), stop=(k == 1))
    # copy psum fsrc rows -> sbuf (partition-aligned; rows 0 and 32)
    fr = pers.tile([33, NH], f32, name="fr")
    nc.vector.tensor_copy(fr[0:1, :], pbig[0:1, 2048:2048 + NH])
    nc.scalar.activation(fr[32:33, :], pbig[32:33, 2048:2048 + NH], AF.Copy)

    # Whaug = x @ [W | wsrc | wdst] -> per i-tile [128, D+2]
    whl = []
    fsd = pers.tile([128, 2 * NT], f32, name="fsd")
    for it in range(NT):
        pwh = pbig[0:128, (it % 4) * 512:(it % 4) * 512 + D + 2]
        for k in range(2):
            nc.tensor.matmul(pwh, lhsT=xt[k][:, it * 128:(it + 1) * 128],
                             rhs=wf[k][:], start=(k == 0), stop=(k == 1))
        t = pers.tile([128, D + 1], bf, name=f"whl{it}")
        nc.vector.tensor_copy(t[:, 0:D], pwh[:, 0:D])
        nc.gpsimd.memset(t[:, D:D + 1], 1.0)
        nc.vector.tensor_copy(fsd[:, 2 * it:2 * it + 2], pwh[:, D:D + 2])
        whl.append(t)
    xtp.__exit__(None, None, None)

    # broadcast fsrc to all partitions; exponential factors.
    # NB: partition_broadcast on HW only reads/writes from partition 0, so
    # offset sources are first DMA-shifted to partition 0.
    frb = pers.tile([33, NH], bf, name="frb")
    nc.vector.tensor_copy(frb[0:1, :], fr[0:1, :])
    nc.vector.tensor_copy(frb[32:33, :], fr[32:33, :])
    frb2 = pers.tile([1, NH], bf, name="frb2")
    nc.sync.dma_start(frb2[0:1, :], frb[32:33, :])
    fsb = pers.tile([128, n], bf, name="fsb")
    nc.gpsimd.partition_broadcast(fsb[:, 0:NH], frb[0:1, :])
    nc.gpsimd.partition_broadcast(fsb[:, NH:n], frb2[0:1, :])
    Ab = pers.tile([128, n], bf, name="Ab")
    Cb = pers.tile([128, n], bf, name="Cb")
    nc.scalar.activation(Ab[:], fsb[:], AF.Exp)
    nc.scalar.activation(Cb[:], fsb[:], AF.Exp, scale=SLOPE)
    fsdr = fsd.rearrange("p (t two) -> p t two", two=2)
    Bc = pers.tile([128, NT], f32, name="Bc")
    Dc = pers.tile([128, NT], f32, name="Dc")
    nfd = pers.tile([128, NT], f32, name="nfd")
    Bcr = Bc.rearrange("p (t o) -> p t o", o=1)
    Dcr = Dc.rearrange("p (t o) -> p t o", o=1)
    nfdr = nfd.rearrange("p (t o) -> p t o", o=1)
    nc.scalar.activation(Bcr[:], fsdr[:, :, 1:2], AF.Exp)
    nc.scalar.activation(Dcr[:], fsdr[:, :, 1:2], AF.Exp, scale=SLOPE)
    nc.vector.tensor_scalar(out=nfdr[:], in0=fsdr[:, :, 1:2], scalar1=-1.0,
                            scalar2=None, op0=OP.mult)

    # woaug = [Wo_h | Wo_h@ao_src | Wo_h@ao_dst]  [D, C+2]
    woTt = pers.tile([128, D], bf, name="woTt")
    nc.gpsimd.memset(woTt[:], 0.0)
    nc.sync.dma_start(woTt[0:C, :], woT_d[:])
    ao2t = pers.tile([128, 2], bf, name="ao2t")
    nc.gpsimd.memset(ao2t[:], 0.0)
    nc.sync.dma_start(ao2t[0:C, :], ao2_d[:])
    # woaug duplicated on partitions 0:64 and 64:128 (matmul requires
    # lhsT/rhs base partitions to match; eluO halves live at 0 and 64)
    woaug = pers.tile([128, C + 2], bf, name="woaug")
    for half in range(2):
        pwo = pbig[half * 64:half * 64 + D, 0:2]
        nc.tensor.matmul(pwo, lhsT=woTt[:, 0:D], rhs=ao2t[:],
                         start=True, stop=True)
        nc.sync.dma_start(woaug[half * 64:half * 64 + D, 0:C], wo_d[:])
        nc.vector.tensor_copy(woaug[half * 64:half * 64 + D, C:C + 2], pwo)

    I128 = pers.tile([128, 128], f32, name="I128")
    make_identity(nc, I128[:])

    # ---- layer-1 j-loop ----
    # ACT-pipe: t = relu(-s) [ACT], u = exp(.8 t) [ACT], v = A*B [DVE ts],
    #           w = u*v [tt], P = w*adj [tt]   (u*v == exp(leaky(s)))
    # DVE-pipe: t1 = A*B [ts], t2 = C*D [ts], w = max(t1,t2) [tt], P = w*adj [tt]
    adj_pool = es.enter_context(tc.tile_pool(name="adj", bufs=2))
    t_pool = es.enter_context(tc.tile_pool(name="t1", bufs=2))
    u_pool = es.enter_context(tc.tile_pool(name="t2", bufs=2))
    p_pool = es.enter_context(tc.tile_pool(name="pp", bufs=2))

    def act_tile(t):
        # spread ACT-pipe tiles among DVE-pipe tiles so both pipelines run
        # concurrently instead of serializing in bursts
        return (t * ACT1_PER16) % 16 < ACT1_PER16

    nchunk = n // 512
    gps_turn = [0]

    def tt_op(out, in0, in1, op, force=None):
        # gpsimd ucode only handles mult/add tensor_tensor; max must go to DVE
        eng = force or ("gps" if gps_turn[0] % 2 == 0 else "dve")
        gps_turn[0] += 1
        if op == OP.max or not USE_GPS:
            eng = "dve"
        if eng == "gps":
            nc.gpsimd.tensor_tensor(out, in0, in1, op)
        else:
            nc.vector.tensor_tensor(out, in0, in1, op)

    for t in range(NT):
        adjt = adj_pool.tile([128, n], bf, tag="adjt")
        nc.sync.dma_start(adjt[:], adjT_d[t * 128:(t + 1) * 128, :])
        P = p_pool.tile([128, n], bf, tag="P")
        if (t % 16) < ACT1_PER16:
            tt1 = t_pool.tile([128, n], bf, tag="tt1")
            nc.scalar.activation(tt1[:], fsb[:], AF.Relu, scale=-1.0,
                                 bias=nfd[:, t:t + 1])
            uu = u_pool.tile([128, n], bf, tag="uu")
            nc.scalar.activation(uu[:], tt1[:], AF.Exp, scale=0.8)
            vv = t_pool.tile([128, n], bf, tag="tt1")
            nc.vector.tensor_scalar(out=vv[:], in0=Ab[:], scalar1=Bc[:, t:t + 1],
                                    scalar2=None, op0=OP.mult)
            ww = u_pool.tile([128, n], bf, tag="uu")
            tt_op(ww[:], uu[:], vv[:], OP.mult)
            tt_op(P[:], ww[:], adjt[:], OP.mult)
        else:
            tt1 = t_pool.tile([128, n], bf, tag="tt1")
            nc.vector.tensor_scalar(out=tt1[:], in0=Ab[:], scalar1=Bc[:, t:t + 1],
                                    scalar2=None, op0=OP.mult)
            uu = u_pool.tile([128, n], bf, tag="uu")
            nc.vector.tensor_scalar(out=uu[:], in0=Cb[:], scalar1=Dc[:, t:t + 1],
                                    scalar2=None, op0=OP.mult)
            ww = t_pool.tile([128, n], bf, tag="tt1")
            tt_op(ww[:], tt1[:], uu[:], OP.max)
            tt_op(P[:], ww[:], adjt[:], OP.mult)
        for c in range(nchunk):
            nc.tensor.matmul(pbig[0:D + 1, c * 512:(c + 1) * 512],
                             lhsT=whl[t][:], rhs=P[:, c * 512:(c + 1) * 512],
                             start=(t == 0), stop=(t == NT - 1))

    # ---- layer-1 epilogue: normalize + elu (split [128, NH] layout) ----
    # psum -> sbuf (partition-aligned compute copies on two engines)
    o1lo = pers.tile([D + 1, NH], f32, name="o1lo")
    o1hi = pers.tile([D + 1, NH], f32, name="o1hi")
    nc.vector.tensor_copy(o1lo[:], pbig[0:D + 1, 0:NH])
    nc.scalar.activation(o1hi[:], pbig[0:D + 1, NH:n], AF.Copy)
    # sbuf->sbuf DMAs to fold into a [128, NH] split layout
    o1s = pers.tile([128, NH], f32, name="o1s")
    nc.sync.dma_start(o1s[0:D, :], o1lo[0:D, :])
    nc.sync.dma_start(o1s[D:2 * D, :], o1hi[0:D, :])
    # 1/Z in place on the Z rows, shift to partition 0, then broadcast.
    # o1hi rows 0:64 are dead once o1s is filled, so they host the second
    # broadcast; zl0 is reused for both Z rows (serialized by Tile deps).
    nc.vector.reciprocal(o1lo[D:D + 1, :], o1lo[D:D + 1, :])
    nc.vector.reciprocal(o1hi[D:D + 1, :], o1hi[D:D + 1, :])
    zl0 = pers.tile([1, NH], f32, name="zl0")
    zb = pers.tile([128, NH], f32, name="zb")
    nc.sync.dma_start(zl0[0:1, :], o1lo[D:D + 1, :])
    nc.gpsimd.partition_broadcast(zb[0:D, :], zl0[0:1, :])
    nc.sync.dma_start(zl0[0:1, :], o1hi[D:D + 1, :])
    nc.gpsimd.partition_broadcast(o1hi[0:D, :], zl0[0:1, :])
    nc.sync.dma_start(zb[D:2 * D, :], o1hi[0:D, :])
    o1n = pers.tile([128, NH], bf, name="o1n")
    nc.vector.tensor_tensor(o1n[:], o1s[:], zb[:], OP.mult)
    # elu
    mm = pers.tile([128, NH], bf, name="mm")
    nc.vector.tensor_scalar(out=mm[:], in0=o1n[:], scalar1=0.0, scalar2=None,
                            op0=OP.min)
    em = pers.tile([128, NH], bf, name="em")
    nc.scalar.activation(em[:], mm[:], AF.Exp)
    r1 = pers.tile([128, NH], bf, name="r1")
    nc.vector.tensor_scalar(out=r1[:], in0=o1n[:], scalar1=0.0, scalar2=-1.0,
                            op0=OP.max, op1=OP.add)
    eluO = pers.tile([128, NH], bf, name="eluO")
    nc.vector.tensor_tensor(eluO[:], r1[:], em[:], OP.add)

    # partial2 = eluO^T.T @ woaug -> [n, C+2]; DMA to rs_in
    g_pool = es.enter_context(tc.tile_pool(name="gin", bufs=4))
    half_t = NT // 2
    for it in range(NT):
        prt = (it // half_t) * D
        col = (it % half_t) * 128
        pt2 = pbig[0:128, (it % 8) * 512:(it % 8) * 512 + C + 2]
        nc.tensor.matmul(pt2, lhsT=eluO[prt:prt + D, col:col + 128],
                         rhs=woaug[prt:prt + D, :], start=True, stop=True)
        gt = g_pool.tile([128, C + 2], bf, tag="gt")
        nc.vector.tensor_copy(gt[:], pt2)
        nc.sync.dma_start(rs_in[it * 128:(it + 1) * 128, :], gt[:])

    # ---- collectives: RS then AG (bf16) ----
    nc.gpsimd.collective_compute("ReduceScatter", mybir.AluOpType.add,
                                 replica_groups=RG, ins=[rs_in.ap().opt()],
                                 outs=[rs_out.ap().opt()])
    nc.gpsimd.collective_compute("AllGather", mybir.AluOpType.bypass,
                                 replica_groups=RG, ins=[rs_out.ap().opt()],
                                 outs=[ag_out.ap().opt()])

    # ---- layer-2 prep ----
    whol = pers.tile([128, NT, C + 1], bf, name="whol")
    for t in range(NT):
        nc.sync.dma_start(whol[:, t, 0:C], ag_out[t * 128:(t + 1) * 128, 0:C])
    nc.gpsimd.memset(whol[:, :, C:C + 1], 1.0)
    gdc = pers.tile([128, NT], bf, name="gdc")
    nc.sync.dma_start(
        gdc[:], ag_out[:, C + 1:C + 2].rearrange("(t p) o -> p (t o)", p=128))
    gdf = pers.tile([128, NT], f32, name="gdf")
    nc.vector.tensor_copy(gdf[:], gdc[:])
    B2c = pers.tile([128, NT], f32, name="B2c")
    D2c = pers.tile([128, NT], f32, name="D2c")
    ngd = pers.tile([128, NT], f32, name="ngd")
    nc.scalar.activation(B2c[:], gdf[:], AF.Exp)
    nc.scalar.activation(D2c[:], gdf[:], AF.Exp, scale=SLOPE)
    nc.vector.tensor_scalar(out=ngd[:], in0=gdf[:], scalar1=-1.0,
                            scalar2=None, op0=OP.mult)
    gsr = pers.tile([1, shard], bf, name="gsr")
    nc.sync.dma_start(
        gsr[:], rs_out[:, C:C + 1].rearrange("(o s) one -> o (s one)", o=1))
    gsb = pers.tile([128, shard], bf, name="gsb")
    nc.gpsimd.partition_broadcast(gsb[:], gsr[0:1, :])
    A2b = pers.tile([128, shard], bf, name="A2b")
    C2b = pers.tile([128, shard], bf, name="C2b")
    nc.scalar.activation(A2b[:], gsb[:], AF.Exp)
    nc.scalar.activation(C2b[:], gsb[:], AF.Exp, scale=SLOPE)

    # ---- layer-2 j-loop ----
    adj2_pool = es.enter_context(tc.tile_pool(name="adj2", bufs=3))
    p2_pool = es.enter_context(tc.tile_pool(name="pp2", bufs=3))
    for t in range(NT):
        adjs = adj2_pool.tile([128, shard], bf, tag="adjs")
        nc.sync.dma_start(adjs[:], adjs_d[t * 128:(t + 1) * 128, :])
        P2 = p2_pool.tile([128, shard], bf, tag="P2")
        if (t % 16) < ACT2_PER16:
            q1 = t_pool.tile([128, shard], bf, tag="q1")
            nc.scalar.activation(q1[:], gsb[:], AF.Relu, scale=-1.0,
                                 bias=ngd[:, t:t + 1])
            q2 = u_pool.tile([128, shard], bf, tag="q2")
            nc.scalar.activation(q2[:], q1[:], AF.Exp, scale=0.8)
            q3 = t_pool.tile([128, shard], bf, tag="q1")
            nc.vector.tensor_scalar(out=q3[:], in0=A2b[:], scalar1=B2c[:, t:t + 1],
                                    scalar2=None, op0=OP.mult)
            q4 = u_pool.tile([128, shard], bf, tag="q2")
            tt_op(q4[:], q2[:], q3[:], OP.mult)
            tt_op(P2[:], q4[:], adjs[:], OP.mult)
        else:
            q1 = t_pool.tile([128, shard], bf, tag="q1")
            nc.vector.tensor_scalar(out=q1[:], in0=A2b[:], scalar1=B2c[:, t:t + 1],
                                    scalar2=None, op0=OP.mult)
            q2 = u_pool.tile([128, shard], bf, tag="q2")
            nc.vector.tensor_scalar(out=q2[:], in0=C2b[:], scalar1=D2c[:, t:t + 1],
                                    scalar2=None, op0=OP.mult)
            q3 = t_pool.tile([128, shard], bf, tag="q1")
            tt_op(q3[:], q1[:], q2[:], OP.max)
            tt_op(P2[:], q3[:], adjs[:], OP.mult)
        nc.tensor.matmul(pbig[0:C + 1, 0:shard], lhsT=whol[:, t, :], rhs=P2[:],
                         start=(t == 0), stop=(t == NT - 1))

    if K_DEBUG:
        tap_fsd = nc.dram_tensor("tap_fsd", [128, 2 * NT], f32, kind="ExternalOutput")
        nc.sync.dma_start(tap_fsd.ap(), fsd[:])
        tap_o1lo = nc.dram_tensor("tap_o1lo", [D + 1, NH], f32, kind="ExternalOutput")
        nc.sync.dma_start(tap_o1lo.ap(), o1lo[:])
        tap_eluO = nc.dram_tensor("tap_eluO", [128, NH], bf, kind="ExternalOutput")
        nc.sync.dma_start(tap_eluO.ap(), eluO[:])
        tap_rsin = nc.dram_tensor("tap_rsin", [n, C + 2], bf, kind="ExternalOutput")
        nc.sync.dma_start(tap_rsin.ap(), rs_in.ap())
        tap_ag = nc.dram_tensor("tap_ag", [n, C + 2], bf, kind="ExternalOutput")
        nc.sync.dma_start(tap_ag.ap(), ag_out.ap())
        tap_gsb = nc.dram_tensor("tap_gsb", [128, shard], bf, kind="ExternalOutput")
        nc.sync.dma_start(tap_gsb.ap(), gsb[:])
        tap_gdf = nc.dram_tensor("tap_gdf", [128, NT], f32, kind="ExternalOutput")
        nc.sync.dma_start(tap_gdf.ap(), gdf[:])

    # ---- layer-2 epilogue: transpose, normalize, elu, log_softmax ----
    o2t = pers.tile([C + 1, shard], f32, name="o2t")
    nc.vector.tensor_copy(o2t[:], pbig[0:C + 1, 0:shard])
    if K_DEBUG:
        tap_o2t = nc.dram_tensor("tap_o2t", [C + 1, shard], f32, kind="ExternalOutput")
        nc.sync.dma_start(tap_o2t.ap(), o2t[:])
    nst = (shard + 127) // 128
    for k in range(nst):
        w = min(128, shard - k * 128)
        ptr = pbig[0:w, 512 + k * 512:512 + k * 512 + C + 1]
        nc.tensor.transpose(ptr, o2t[:, k * 128:k * 128 + w],
                            I128[0:C + 1, 0:C + 1])
        zr = pers.tile([128, 1], f32, name=f"zr{k}")
        nc.vector.reciprocal(zr[0:w, :], ptr[:, C:C + 1])
        o2n = pers.tile([128, C], f32, name=f"o2n{k}")
        nc.vector.tensor_scalar(out=o2n[0:w, :], in0=ptr[:, 0:C],
                                scalar1=zr[0:w, :], scalar2=None, op0=OP.mult)
        m2 = pers.tile([128, C], f32, name=f"m2{k}")
        nc.vector.tensor_scalar(out=m2[0:w, :], in0=o2n[0:w, :], scalar1=0.0,
                                scalar2=None, op0=OP.min)
        e2 = pers.tile([128, C], f32, name=f"e2{k}")
        nc.scalar.activation(e2[0:w, :], m2[0:w, :], AF.Exp)
        r2 = pers.tile([128, C], f32, name=f"r2{k}")
        nc.vector.tensor_scalar(out=r2[0:w, :], in0=o2n[0:w, :], scalar1=0.0,
                                scalar2=-1.0, op0=OP.max, op1=OP.add)
        el2 = pers.tile([128, C], f32, name=f"el2{k}")
        nc.vector.tensor_tensor(el2[0:w, :], r2[0:w, :], e2[0:w, :], OP.add)
        # log_softmax over free axis
        mx = pers.tile([128, 1], f32, name=f"mx{k}")
        nc.vector.tensor_reduce(mx[0:w, :], el2[0:w, :],
                                mybir.AxisListType.X, OP.max)
        xm = pers.tile([128, C], f32, name=f"xm{k}")
        nc.vector.tensor_scalar(out=xm[0:w, :], in0=el2[0:w, :],
                                scalar1=mx[0:w, :], scalar2=None,
                                op0=OP.subtract)
        ex = pers.tile([128, C], f32, name=f"ex{k}")
        sume = pers.tile([128, 1], f32, name=f"sume{k}")
        nc.scalar.activation(ex[0:w, :], xm[0:w, :], AF.Exp,
                             accum_out=sume[0:w, :])
        lns = pers.tile([128, 1], f32, name=f"lns{k}")
        nc.scalar.activation(lns[0:w, :], sume[0:w, :], AF.Ln)
        ok = pers.tile([128, C], f32, name=f"ok{k}")
        nc.vector.tensor_scalar(out=ok[0:w, :], in0=xm[0:w, :],
                                scalar1=lns[0:w, :], scalar2=None,
                                op0=OP.subtract)
        nc.sync.dma_start(out_d[k * 128:k * 128 + w, :], ok[0:w, :])

    es.close()


def build(n=N, debug=False):
    from concourse import bacc
    import concourse.tile as tile

    nc = bacc.Bacc("TRN2", target_bir_lowering=False, debug=debug,
                   num_devices=CORES)
    with tile.TileContext(nc) as tc:
        _emit(nc, tc, n, n // CORES)
    nc.compile()
    return nc


def make_in_maps(x, adj, W, a, Wo, ao, n=N):
    """Host-side shard/layout prep -> list of 8 input dicts."""
    shard = n // CORES
    xT = np.ascontiguousarray(x.T).astype(BF)
    adjT = np.ascontiguousarray(adj.T).astype(BF)
    in_maps = []
    for h in range(CORES):
        wh = W[h].astype(BF)
        woh = Wo[h * D:(h + 1) * D, :].astype(BF)
        in_maps.append({
            "xT": xT,
            "adjT": adjT,
            "adjs": np.ascontiguousarray(adjT[:, h * shard:(h + 1) * shard]),
            "wh": wh,
            "whT": np.ascontiguousarray(wh.T),
            "a2": np.ascontiguousarray(np.stack([a[h, :D], a[h, D:]], axis=1)).astype(BF),
            "wo": woh,
            "woT": np.ascontiguousarray(woh.T),
            "ao2": np.ascontiguousarray(np.stack([ao[:C], ao[C:]], axis=1)).astype(BF),
        })
    return in_maps


def kernel(x, adj, W, a, Wo, ao):
    from concourse.bass_utils import run_bass_kernel_spmd

    x = np.asarray(x, np.float32)
    adj = np.asarray(adj, np.float32)
    W = np.asarray(W, np.float32)
    a = np.asarray(a, np.float32)
    Wo = np.asarray(Wo, np.float32)
    ao = np.asarray(ao, np.float32)

    if "nc" not in _BASS_CACHE:
        _BASS_CACHE["nc"] = build()
    nc = _BASS_CACHE["nc"]
    in_maps = make_in_maps(x, adj, W, a, Wo, ao)
    r = run_bass_kernel_spmd(nc, in_maps, core_ids=list(range(CORES)))
    out = np.concatenate([r.results[c]["out"] for c in range(CORES)], axis=0)
    return np.asarray(out, np.float32)



# revision 13
# speedup vs baseline: 1.4494x; 1.4494x over previous
"""Trainium2 Bass kernel for a 2-layer GAT (nn_AGAEMD problem).

Sharding: layer-1 heads across 8 cores (core h owns head h, full N x N
attention for that head); layer-2 row-sharded (core c owns output rows
[c*512, (c+1)*512)).  Head outputs are combined with ONE bf16 AllReduce
over a flat contiguous payload (Who partials + a ones column + gT rows);
the per-core g_src slice is extracted post-AR with a one-hot selection
matmul (rsel input), avoiding any core-dependent addressing.

Math notes:
 - softmax rows are invariant to any per-column factor, so instead of
   P = exp(leaky(fs_i + fd_j))*adj we compute
   G2 = exp(0.8*relu(s) + 0.2*fd_j)*adj  (= P * exp(-0.2*fs_i)),
   which normalizes to the same attention.  Two equivalent pipelines:
     ACT-form: t1 = Relu(fsb + fd_j), t2 = Exp(0.8*t1 + 0.2*fd_j), mask
     DVE-form: u = A8b * B_j (ts), w = max(u, D_j) (ts), mask
   with A8b = exp(0.8*fs_i) broadcast, B = exp(fd), D = exp(0.2*fd).
 - reciprocals are computed as exp(-ln(x)) on the scalar engine (the
   DVE RECIPROCAL instruction costs ~5.3us regardless of size).
 - elu(x) = max(x,0) - 1 + exp(min(x,0)).
"""

import sys

if "/opt/trn_rl_repo" not in sys.path:
    sys.path.insert(0, "/opt/trn_rl_repo")

import numpy as np
import ml_dtypes

BF = ml_dtypes.bfloat16

# problem dims (hardcoded per spec)
N, F, H, D, C = 4096, 256, 8, 64, 64
CORES = 8
SLOPE = 0.2

import os as _os

# engine-split tunables: #ACT-form tiles (of 32) and #mask ops on gpsimd
ACT1_N = int(_os.environ.get("K_ACT1", "13"))
GPS1_N = int(_os.environ.get("K_GPS1", "12"))
ACT2_N = int(_os.environ.get("K_ACT2", "13"))
GPS2_N = int(_os.environ.get("K_GPS2", "10"))
K_DEBUG = int(_os.environ.get("K_DEBUG", "0"))

_BASS_CACHE = {}


def _spread(k, nt):
    """k tile indices spread evenly over range(nt) (Bresenham)."""
    return {t for t in range(nt) if ((t + 1) * k) // nt > (t * k) // nt}


def _emit(nc, tc, n, shard):
    """Emit the SPMD per-core graph. n = graph size (4096 full), shard = n//8."""
    import concourse.bass as bass
    import concourse.mybir as mybir
    from concourse.masks import make_identity

    bf = mybir.dt.bfloat16
    f32 = mybir.dt.float32
    AF = mybir.ActivationFunctionType
    OP = mybir.AluOpType
    NT = n // 128          # number of 128-row tiles
    NH = n // 2            # split-layout free width
    RG = [list(range(CORES))]
    C1 = C + 1             # who payload row: C cols + ones col

    # ---- dram I/O ----
    xT_d = nc.dram_tensor("xT", [F, n], bf, kind="ExternalInput")
    adjT_d = nc.dram_tensor("adjT", [n, n], bf, kind="ExternalInput")
    adjs_d = nc.dram_tensor("adjs", [n, shard], bf, kind="ExternalInput")
    wh_d = nc.dram_tensor("wh", [F, D], bf, kind="ExternalInput")
    whT_d = nc.dram_tensor("whT", [D, F], bf, kind="ExternalInput")
    a2_d = nc.dram_tensor("a2", [D, 2], bf, kind="ExternalInput")
    wo_d = nc.dram_tensor("wo", [D, C], bf, kind="ExternalInput")
    woT_d = nc.dram_tensor("woT", [C, D], bf, kind="ExternalInput")
    ao2_d = nc.dram_tensor("ao2", [C, 2], bf, kind="ExternalInput")
    rsel_d = nc.dram_tensor("rsel", [CORES, 1], bf, kind="ExternalInput")
    out_d = nc.dram_tensor("out", [shard, C], f32, kind="ExternalOutput")

    # collective bounce buffers: flat payload = [n, C1] who rows + [2, n] gT
    FLAT = n * C1 + 2 * n
    rs_in = nc.dram_tensor("rs_in", [1, FLAT], bf)
    ag_out = nc.dram_tensor("ag_out", [1, FLAT], bf, addr_space="Shared")
    who_w = rs_in.ap()[0:1, 0:n * C1].rearrange("one (r c) -> (one r) c", c=C1)
    g_w = rs_in.ap()[0:1, n * C1:FLAT].rearrange("one (g i) -> (one g) i", i=n)
    who_r = ag_out.ap()[0:1, 0:n * C1].rearrange("one (r c) -> (one r) c", c=C1)
    g_r = ag_out.ap()[0:1, n * C1:FLAT].rearrange("one (g i) -> (one g) i", i=n)

    from contextlib import ExitStack

    es = ExitStack()
    pers = es.enter_context(tc.tile_pool(name="pers", bufs=1))
    ppool = es.enter_context(tc.tile_pool(name="psum", bufs=1, space="PSUM"))
    pbig = ppool.tile([128, 4096], f32, name="pbig")

    # ---- prologue: weights ----
    xtp = tc.tile_pool(name="xtp", bufs=1)
    xtpool = xtp.__enter__()
    xt = []
    for k in range(2):
        t = xtpool.tile([128, n], bf, name=f"xt{k}")
        nc.sync.dma_start(t[:], xT_d[k * 128:(k + 1) * 128, :])
        xt.append(t)
    wf = []
    for k in range(2):
        t = pers.tile([128, D + 2], bf, name=f"wf{k}")
        nc.sync.dma_start(t[:, 0:D], wh_d[k * 128:(k + 1) * 128, :])
        wf.append(t)
    whTt = pers.tile([128, F], bf, name="whTt")
    nc.gpsimd.memset(whTt[:], 0.0)
    nc.sync.dma_start(whTt[0:D, :], whT_d[:])
    a2t = pers.tile([128, 2], bf, name="a2t")
    nc.gpsimd.memset(a2t[:], 0.0)
    nc.sync.dma_start(a2t[0:D, :], a2_d[:])
    rselt = pers.tile([CORES, 1], bf, name="rselt")
    nc.sync.dma_start(rselt[:], rsel_d[:])

    # waug = W_h @ a2 : [F, 2] (two 128-row tiles)
    for k in range(2):
        pw = pbig[0:128, k * 512:k * 512 + 2]
        nc.tensor.matmul(pw, lhsT=whTt[:, k * 128:(k + 1) * 128], rhs=a2t[:],
                         start=True, stop=True)
        nc.vector.tensor_copy(wf[k][:, D:D + 2], pw)

    # fsrc row via matmul: fsrc = x @ wsrc -> psum rows, split on partitions 0/32
    n_cc = n // 512
    cpg = max(1, NH // 512)  # 512-chunks per half
    for cc in range(n_cc):
        part = (cc // cpg) * 32  # matmul out base partition must be 0/32/64
        foff = 2048 + 512 * (cc % cpg)
        pr = pbig[part:part + 1, foff:foff + 512]
        for k in range(2):
            nc.tensor.matmul(pr, lhsT=wf[k][:, D:D + 1],
                             rhs=xt[k][:, cc * 512:(cc + 1) * 512],
                             start=(k == 0), stop=(k == 1))
    # copy psum fsrc rows -> sbuf (partition-aligned; rows 0 and 32)
    fr = xtpool.tile([33, NH], f32, name="fr")
    nc.vector.tensor_copy(fr[0:1, :], pbig[0:1, 2048:2048 + NH])
    nc.scalar.activation(fr[32:33, :], pbig[32:33, 2048:2048 + NH], AF.Copy)

    # Whaug = x @ [W | wsrc | wdst] -> per i-tile [128, D+2]
    whl = []
    fsd = pers.tile([128, 2 * NT], f32, name="fsd")
    for it in range(NT):
        pwh = pbig[0:128, (it % 4) * 512:(it % 4) * 512 + D + 2]
        for k in range(2):
            nc.tensor.matmul(pwh, lhsT=xt[k][:, it * 128:(it + 1) * 128],
                             rhs=wf[k][:], start=(k == 0), stop=(k == 1))
        t = pers.tile([128, D + 1], bf, name=f"whl{it}")
        nc.vector.tensor_copy(t[:, 0:D], pwh[:, 0:D])
        nc.gpsimd.memset(t[:, D:D + 1], 1.0)
        nc.vector.tensor_copy(fsd[:, 2 * it:2 * it + 2], pwh[:, D:D + 2])
        whl.append(t)

    # broadcast fsrc to all partitions; A8b = exp(0.8*fs_i).
    # NB: partition_broadcast on HW only reads from partition 0, so the
    # offset source row is first DMA-shifted to partition 0.
    frb = xtpool.tile([33, NH], bf, name="frb")
    nc.vector.tensor_copy(frb[0:1, :], fr[0:1, :])
    nc.vector.tensor_copy(frb[32:33, :], fr[32:33, :])
    frb2 = xtpool.tile([1, NH], bf, name="frb2")
    nc.sync.dma_start(frb2[0:1, :], frb[32:33, :])
    fsb = pers.tile([128, n], bf, name="fsb")
    nc.gpsimd.partition_broadcast(fsb[:, 0:NH], frb[0:1, :])
    nc.gpsimd.partition_broadcast(fsb[:, NH:n], frb2[0:1, :])
    A8b = pers.tile([128, n], bf, name="A8b")
    nc.scalar.activation(A8b[:], fsb[:], AF.Exp, scale=0.8)
    xtp.__exit__(None, None, None)

    # per-partition fd constants: raw fd, 0.2*fd, exp(fd), exp(0.2*fd)
    fsdr = fsd.rearrange("p (t two) -> p t two", two=2)
    fdc = pers.tile([128, NT], f32, name="fdc")
    fd02 = pers.tile([128, NT], f32, name="fd02")
    Bc = pers.tile([128, NT], f32, name="Bc")
    Dc = pers.tile([128, NT], f32, name="Dc")
    fdcr = fdc.rearrange("p (t o) -> p t o", o=1)
    fd02r = fd02.rearrange("p (t o) -> p t o", o=1)
    Bcr = Bc.rearrange("p (t o) -> p t o", o=1)
    Dcr = Dc.rearrange("p (t o) -> p t o", o=1)
    nc.vector.tensor_copy(fdcr[:], fsdr[:, :, 1:2])
    nc.vector.tensor_scalar(out=fd02r[:], in0=fsdr[:, :, 1:2], scalar1=SLOPE,
                            scalar2=None, op0=OP.mult)
    nc.scalar.activation(Bcr[:], fsdr[:, :, 1:2], AF.Exp)
    nc.scalar.activation(Dcr[:], fsdr[:, :, 1:2], AF.Exp, scale=SLOPE)

    # woaug = [Wo_h | Wo_h@ao_src | Wo_h@ao_dst]  [D, C+2], duplicated on
    # partitions 0:64 and 64:128 (matmul requires lhsT/rhs base partitions
    # to match; eluO halves live at 0 and 64)
    woTt = pers.tile([128, D], bf, name="woTt")
    nc.gpsimd.memset(woTt[:], 0.0)
    nc.sync.dma_start(woTt[0:C, :], woT_d[:])
    ao2t = pers.tile([128, 2], bf, name="ao2t")
    nc.gpsimd.memset(ao2t[:], 0.0)
    nc.sync.dma_start(ao2t[0:C, :], ao2_d[:])
    woaug = pers.tile([128, C + 2], bf, name="woaug")
    for half in range(2):
        pwo = pbig[half * 64:half * 64 + D, 0:2]
        nc.tensor.matmul(pwo, lhsT=woTt[:, 0:D], rhs=ao2t[:],
                         start=True, stop=True)
        nc.sync.dma_start(woaug[half * 64:half * 64 + D, 0:C], wo_d[:])
        nc.vector.tensor_copy(woaug[half * 64:half * 64 + D, C:C + 2], pwo)

    I128 = pers.tile([128, 128], f32, name="I128")
    make_identity(nc, I128[:])

    # ---- layer-1 j-loop ----
    l1es = ExitStack()
    adj_pool = l1es.enter_context(tc.tile_pool(name="adj", bufs=2))
    t_pool = l1es.enter_context(tc.tile_pool(name="t1", bufs=2))
    u_pool = l1es.enter_context(tc.tile_pool(name="t2", bufs=2))
    p_pool = l1es.enter_context(tc.tile_pool(name="pp", bufs=2))

    act_set = _spread(ACT1_N, NT)
    # masks to gpsimd: prefer ACT-form tiles (their chains avoid DVE)
    order = [t for t in range(NT) if t in act_set] + \
            [t for t in range(NT) if t not in act_set]
    gps_set = set(order[:GPS1_N])

    nchunk = n // 512
    for t in range(NT):
        adjt = adj_pool.tile([128, n], bf, tag="adjt")
        nc.sync.dma_start(adjt[:], adjT_d[t * 128:(t + 1) * 128, :])
        P = p_pool.tile([128, n], bf, tag="P")
        if t in act_set:
            tt1 = t_pool.tile([128, n], bf, tag="tt1")
            nc.scalar.activation(tt1[:], fsb[:], AF.Relu,
                                 bias=fdc[:, t:t + 1])
            uu = u_pool.tile([128, n], bf, tag="uu")
            nc.scalar.activation(uu[:], tt1[:], AF.Exp, scale=0.8,
                                 bias=fd02[:, t:t + 1])
        else:
            tt1 = t_pool.tile([128, n], bf, tag="tt1")
            nc.vector.tensor_scalar(out=tt1[:], in0=A8b[:],
                                    scalar1=Bc[:, t:t + 1],
                                    scalar2=None, op0=OP.mult)
            uu = u_pool.tile([128, n], bf, tag="uu")
            nc.vector.tensor_scalar(out=uu[:], in0=tt1[:],
                                    scalar1=Dc[:, t:t + 1],
                                    scalar2=None, op0=OP.max)
        if t in gps_set:
            nc.gpsimd.tensor_tensor(P[:], uu[:], adjt[:], OP.mult)
        else:
            nc.vector.tensor_tensor(P[:], uu[:], adjt[:], OP.mult)
        for c in range(nchunk):
            nc.tensor.matmul(pbig[0:D + 1, c * 512:(c + 1) * 512],
                             lhsT=whl[t][:], rhs=P[:, c * 512:(c + 1) * 512],
                             start=(t == 0), stop=(t == NT - 1))
    l1es.close()

    # ---- layer-1 epilogue: normalize + elu (split [128, NH] layout) ----
    # psum -> sbuf (partition-aligned compute copies on two engines);
    # transient tiles live in a scoped pool freed before layer-2 prep
    epp = tc.tile_pool(name="epp", bufs=1)
    ep = epp.__enter__()
    o1lo = ep.tile([D + 1, NH], f32, name="o1lo")
    o1hi = ep.tile([D + 1, NH], f32, name="o1hi")
    nc.vector.tensor_copy(o1lo[:], pbig[0:D + 1, 0:NH])
    nc.scalar.activation(o1hi[:], pbig[0:D + 1, NH:n], AF.Copy)
    # sbuf->sbuf DMAs to fold into a [128, NH] split layout
    o1s = ep.tile([128, NH], f32, name="o1s")
    nc.sync.dma_start(o1s[0:D, :], o1lo[0:D, :])
    nc.sync.dma_start(o1s[D:2 * D, :], o1hi[0:D, :])
    # 1/Z via exp(-ln(Z)) on the scalar engine (DVE reciprocal has a
    # ~5.3us fixed cost): shift Z rows to partition 0, invert, broadcast.
    zl0f = ep.tile([1, NH], f32, name="zl0f")
    zl0 = ep.tile([1, NH], bf, name="zl0")
    zb = ep.tile([128, NH], bf, name="zb")
    zbx = ep.tile([D, NH], bf, name="zbx")
    nc.sync.dma_start(zl0f[0:1, :], o1lo[D:D + 1, :])
    nc.scalar.activation(zl0f[0:1, :], zl0f[0:1, :], AF.Ln)
    nc.scalar.activation(zl0[0:1, :], zl0f[0:1, :], AF.Exp, scale=-1.0)
    nc.gpsimd.partition_broadcast(zb[0:D, :], zl0[0:1, :])
    nc.sync.dma_start(zl0f[0:1, :], o1hi[D:D + 1, :])
    nc.scalar.activation(zl0f[0:1, :], zl0f[0:1, :], AF.Ln)
    nc.scalar.activation(zl0[0:1, :], zl0f[0:1, :], AF.Exp, scale=-1.0)
    nc.gpsimd.partition_broadcast(zbx[0:D, :], zl0[0:1, :])
    nc.sync.dma_start(zb[D:2 * D, :], zbx[0:D, :])
    o1n = ep.tile([128, NH], bf, name="o1n")
    nc.vector.tensor_tensor(o1n[:], o1s[:], zb[:], OP.mult)
    # elu
    mm = ep.tile([128, NH], bf, name="mm")
    nc.vector.tensor_scalar(out=mm[:], in0=o1n[:], scalar1=0.0, scalar2=None,
                            op0=OP.min)
    em = ep.tile([128, NH], bf, name="em")
    nc.scalar.activation(em[:], mm[:], AF.Exp)
    r1 = ep.tile([128, NH], bf, name="r1")
    nc.vector.tensor_scalar(out=r1[:], in0=o1n[:], scalar1=0.0, scalar2=-1.0,
                            op0=OP.max, op1=OP.add)
    eluO = pers.tile([128, NH], bf, name="eluO")
    nc.vector.tensor_tensor(eluO[:], r1[:], em[:], OP.add)

    # partial Who = eluO^T.T @ wo -> [n, C] into gts (col C holds 1/8 so the
    # AllReduce sum yields the ones column used for Z2); DMA to who region
    gts = pers.tile([128, NT, C1], bf, name="gts")
    nc.gpsimd.memset(gts[:, :, C:C + 1], 1.0 / CORES)
    half_t = NT // 2
    for it in range(NT):
        prt = (it // half_t) * D
        col = (it % half_t) * 128
        pt2 = pbig[0:128, (it % 8) * 512:(it % 8) * 512 + C]
        nc.tensor.matmul(pt2, lhsT=eluO[prt:prt + D, col:col + 128],
                         rhs=woaug[prt:prt + D, 0:C], start=True, stop=True)
        nc.vector.tensor_copy(gts[:, it, 0:C], pt2)
        nc.sync.dma_start(who_w[it * 128:(it + 1) * 128, :], gts[:, it, :])

    # gT = [g_src | g_dst]^T as [2, n]: out[r, i] = sum_d ao2[d, r]*eluO[d, i]
    for cc in range(n_cc):
        half = cc // cpg
        col = (cc % cpg) * 512
        pg = pbig[0:2, cc * 512:(cc + 1) * 512]
        nc.tensor.matmul(pg, lhsT=woaug[half * 64:half * 64 + D, C:C + 2],
                         rhs=eluO[half * 64:half * 64 + D, col:col + 512],
                         start=True, stop=True)
    gtt = ep.tile([2, n], bf, name="gtt")
    nc.vector.tensor_copy(gtt[:, 0:NH], pbig[0:2, 0:NH])
    nc.scalar.activation(gtt[:, NH:n], pbig[0:2, NH:n], AF.Copy)
    nc.sync.dma_start(g_w[:], gtt[:])
    epp.__exit__(None, None, None)

    # prefetch layer-2 adj slice (overlaps the collective)
    adjs2 = pers.tile([128, NT * shard], bf, name="adjs2")
    adjs2v = adjs2.rearrange("p (t i) -> p t i", t=NT)
    nc.scalar.dma_start(
        adjs2v[:], adjs_d.ap().rearrange("(t p) i -> p t i", p=128))

    # ---- collective: single AllReduce over the flat payload ----
    nc.gpsimd.collective_compute(
        "AllReduce", mybir.AluOpType.add, replica_groups=RG,
        ins=[rs_in.ap().rearrange("one (a b) -> (one a) b", a=CORES)],
        outs=[ag_out.ap().rearrange("one (a b) -> (one a) b", a=CORES)])

    # ---- layer-2 prep ----
    whol = pers.tile([128, NT, C1], bf, name="whol")
    for t in range(NT):
        nc.sync.dma_start(whol[:, t, :], who_r[t * 128:(t + 1) * 128, :])
    # g_dst -> [128, NT] via partition-split DMA of the flat row
    gdc = pers.tile([128, NT], bf, name="gdc")
    nc.sync.dma_start(
        gdc[:], g_r[1:2, :].rearrange("one (t p) -> (one p) t", p=128))
    gdf = pers.tile([128, NT], f32, name="gdf")
    gd02 = pers.tile([128, NT], f32, name="gd02")
    B2c = pers.tile([128, NT], f32, name="B2c")
    D2c = pers.tile([128, NT], f32, name="D2c")
    nc.vector.tensor_copy(gdf[:], gdc[:])
    nc.vector.tensor_scalar(out=gd02[:], in0=gdc[:], scalar1=SLOPE,
                            scalar2=None, op0=OP.mult)
    nc.scalar.activation(B2c[:], gdc[:], AF.Exp)
    nc.scalar.activation(D2c[:], gdc[:], AF.Exp, scale=SLOPE)
    # g_src slice for this core: one-hot rsel @ g_src viewed as [8, shard]
    gs8 = pers.tile([CORES, shard], bf, name="gs8")
    nc.sync.dma_start(
        gs8[:], g_r[0:1, :].rearrange("one (a i) -> (one a) i", a=CORES))
    pgs = pbig[0:1, 3584:3584 + shard]
    nc.tensor.matmul(pgs, lhsT=rselt[:], rhs=gs8[:], start=True, stop=True)
    gsr = pers.tile([1, shard], bf, name="gsr")
    nc.vector.tensor_copy(gsr[:], pgs)
    gsb = pers.tile([128, shard], bf, name="gsb")
    nc.gpsimd.partition_broadcast(gsb[:], gsr[0:1, :])
    A2b = pers.tile([128, shard], bf, name="A2b")
    nc.scalar.activation(A2b[:], gsb[:], AF.Exp, scale=0.8)

    # ---- layer-2 j-loop ----
    t_pool = es.enter_context(tc.tile_pool(name="t1b", bufs=2))
    u_pool = es.enter_context(tc.tile_pool(name="t2b", bufs=2))
    p_pool = es.enter_context(tc.tile_pool(name="ppb", bufs=2))
    act2_set = _spread(ACT2_N, NT)
    order2 = [t for t in range(NT) if t in act2_set] + \
             [t for t in range(NT) if t not in act2_set]
    gps2_set = set(order2[:GPS2_N])
    for t in range(NT):
        adjs = adjs2[:, t * shard:(t + 1) * shard]
        P2 = p_pool.tile([128, shard], bf, tag="P2")
        if t in act2_set:
            q1 = t_pool.tile([128, shard], bf, tag="q1")
            nc.scalar.activation(q1[:], gsb[:], AF.Relu,
                                 bias=gdf[:, t:t + 1])
            q2 = u_pool.tile([128, shard], bf, tag="q2")
            nc.scalar.activation(q2[:], q1[:], AF.Exp, scale=0.8,
                                 bias=gd02[:, t:t + 1])
        else:
            q1 = t_pool.tile([128, shard], bf, tag="q1")
            nc.vector.tensor_scalar(out=q1[:], in0=A2b[:],
                                    scalar1=B2c[:, t:t + 1],
                                    scalar2=None, op0=OP.mult)
            q2 = u_pool.tile([128, shard], bf, tag="q2")
            nc.vector.tensor_scalar(out=q2[:], in0=q1[:],
                                    scalar1=D2c[:, t:t + 1],
                                    scalar2=None, op0=OP.max)
        if t in gps2_set:
            nc.gpsimd.tensor_tensor(P2[:], q2[:], adjs, OP.mult)
        else:
            nc.vector.tensor_tensor(P2[:], q2[:], adjs, OP.mult)
        nc.tensor.matmul(pbig[0:C + 1, 0:shard], lhsT=whol[:, t, :], rhs=P2[:],
                         start=(t == 0), stop=(t == NT - 1))

    if K_DEBUG:
        tap_fsd = nc.dram_tensor("tap_fsd", [128, 2 * NT], f32, kind="ExternalOutput")
        nc.sync.dma_start(tap_fsd.ap(), fsd[:])
        tap_o1lo = nc.dram_tensor("tap_o1lo", [D + 1, NH], f32, kind="ExternalOutput")
        nc.sync.dma_start(tap_o1lo.ap(), o1lo[:])
        tap_eluO = nc.dram_tensor("tap_eluO", [128, NH], bf, kind="ExternalOutput")
        nc.sync.dma_start(tap_eluO.ap(), eluO[:])
        tap_rsin = nc.dram_tensor("tap_rsin", [1, FLAT], bf, kind="ExternalOutput")
        nc.sync.dma_start(tap_rsin.ap(), rs_in.ap())
        tap_ag = nc.dram_tensor("tap_ag", [1, FLAT], bf, kind="ExternalOutput")
        nc.sync.dma_start(tap_ag.ap(), ag_out.ap())
        tap_gsb = nc.dram_tensor("tap_gsb", [128, shard], bf, kind="ExternalOutput")
        nc.sync.dma_start(tap_gsb.ap(), gsb[:])
        tap_gdf = nc.dram_tensor("tap_gdf", [128, NT], f32, kind="ExternalOutput")
        nc.sync.dma_start(tap_gdf.ap(), gdf[:])

    # ---- layer-2 epilogue: transpose, normalize, elu, log_softmax ----
    o2t = pers.tile([C + 1, shard], f32, name="o2t")
    nc.vector.tensor_copy(o2t[:], pbig[0:C + 1, 0:shard])
    if K_DEBUG:
        tap_o2t = nc.dram_tensor("tap_o2t", [C + 1, shard], f32, kind="ExternalOutput")
        nc.sync.dma_start(tap_o2t.ap(), o2t[:])
    nst = (shard + 127) // 128
    for k in range(nst):
        w = min(128, shard - k * 128)
        ptr = pbig[0:w, 512 + k * 512:512 + k * 512 + C + 1]
        nc.tensor.transpose(ptr, o2t[:, k * 128:k * 128 + w],
                            I128[0:C + 1, 0:C + 1])
        zr = pers.tile([128, 1], f32, name=f"zr{k}")
        nc.scalar.activation(zr[0:w, :], ptr[:, C:C + 1], AF.Ln)
        nc.scalar.activation(zr[0:w, :], zr[0:w, :], AF.Exp, scale=-1.0)
        o2n = pers.tile([128, C], f32, name=f"o2n{k}")
        nc.vector.tensor_scalar(out=o2n[0:w, :], in0=ptr[:, 0:C],
                                scalar1=zr[0:w, :], scalar2=None, op0=OP.mult)
        m2 = pers.tile([128, C], f32, name=f"m2{k}")
        nc.vector.tensor_scalar(out=m2[0:w, :], in0=o2n[0:w, :], scalar1=0.0,
                                scalar2=None, op0=OP.min)
        e2 = pers.tile([128, C], f32, name=f"e2{k}")
        nc.scalar.activation(e2[0:w, :], m2[0:w, :], AF.Exp)
        r2 = pers.tile([128, C], f32, name=f"r2{k}")
        nc.vector.tensor_scalar(out=r2[0:w, :], in0=o2n[0:w, :], scalar1=0.0,
                                scalar2=-1.0, op0=OP.max, op1=OP.add)
        el2 = pers.tile([128, C], f32, name=f"el2{k}")
        nc.vector.tensor_tensor(el2[0:w, :], r2[0:w, :], e2[0:w, :], OP.add)
        # log_softmax over free axis
        mx = pers.tile([128, 1], f32, name=f"mx{k}")
        nc.vector.tensor_reduce(mx[0:w, :], el2[0:w, :],
                                mybir.AxisListType.X, OP.max)
        xm = pers.tile([128, C], f32, name=f"xm{k}")
        nc.vector.tensor_scalar(out=xm[0:w, :], in0=el2[0:w, :],
                                scalar1=mx[0:w, :], scalar2=None,
                                op0=OP.subtract)
        ex = pers.tile([128, C], f32, name=f"ex{k}")
        sume = pers.tile([128, 1], f32, name=f"sume{k}")
        nc.scalar.activation(ex[0:w, :], xm[0:w, :], AF.Exp,
                             accum_out=sume[0:w, :])
        lns = pers.tile([128, 1], f32, name=f"lns{k}")
        nc.scalar.activation(lns[0:w, :], sume[0:w, :], AF.Ln)
        ok = pers.tile([128, C], f32, name=f"ok{k}")
        nc.vector.tensor_scalar(out=ok[0:w, :], in0=xm[0:w, :],
                                scalar1=lns[0:w, :], scalar2=None,
                                op0=OP.subtract)
        nc.sync.dma_start(out_d[k * 128:k * 128 + w, :], ok[0:w, :])

    es.close()


def build(n=N, debug=False):
    from concourse import bacc
    import concourse.tile as tile

    nc = bacc.Bacc("TRN2", target_bir_lowering=False, debug=debug,
                   num_devices=CORES)
    with tile.TileContext(nc) as tc:
        _emit(nc, tc, n, n // CORES)
    nc.compile()
    return nc


def make_in_maps(x, adj, W, a, Wo, ao, n=N):
    """Host-side shard/layout prep -> list of 8 input dicts."""
    shard = n // CORES
    xT = np.ascontiguousarray(x.T).astype(BF)
    adjT = np.ascontiguousarray(adj.T).astype(BF)
    in_maps = []
    for h in range(CORES):
        wh = W[h].astype(BF)
        woh = Wo[h * D:(h + 1) * D, :].astype(BF)
        rsel = np.zeros((CORES, 1), dtype=BF)
        rsel[h, 0] = 1.0
        in_maps.append({
            "xT": xT,
            "adjT": adjT,
            "adjs": np.ascontiguousarray(adjT[:, h * shard:(h + 1) * shard]),
            "wh": wh,
            "whT": np.ascontiguousarray(wh.T),
            "a2": np.ascontiguousarray(np.stack([a[h, :D], a[h, D:]], axis=1)).astype(BF),
            "wo": woh,
            "woT": np.ascontiguousarray(woh.T),
            "ao2": np.ascontiguousarray(np.stack([ao[:C], ao[C:]], axis=1)).astype(BF),
            "rsel": rsel,
        })
    return in_maps


def kernel(x, adj, W, a, Wo, ao):
    from concourse.bass_utils import run_bass_kernel_spmd

    x = np.asarray(x, np.float32)
    adj = np.asarray(adj, np.float32)
    W = np.asarray(W, np.float32)
    a = np.asarray(a, np.float32)
    Wo = np.asarray(Wo, np.float32)
    ao = np.asarray(ao, np.float32)

    if "nc" not in _BASS_CACHE:
        _BASS_CACHE["nc"] = build()
    nc = _BASS_CACHE["nc"]
    in_maps = make_in_maps(x, adj, W, a, Wo, ao)
    r = run_bass_kernel_spmd(nc, in_maps, core_ids=list(range(CORES)))
    out = np.concatenate([r.results[c]["out"] for c in range(CORES)], axis=0)
    return np.asarray(out, np.float32)


# revision 18
# speedup vs baseline: 1.5175x; 1.0470x over previous
"""Trainium2 Bass kernel for a 2-layer GAT (nn_AGAEMD problem).

Sharding: layer-1 heads across 8 cores (core h owns head h, full N x N
attention for that head); layer-2 row-sharded (core c owns output rows
[c*512, (c+1)*512)).  Head outputs are combined with ONE bf16 AllReduce
over a flat contiguous payload (Who partials + a ones column + gT rows);
the per-core g_src slice is extracted post-AR with a one-hot selection
matmul (rsel input), avoiding any core-dependent addressing.

Math notes:
 - softmax rows are invariant to any per-column factor, so instead of
   P = exp(leaky(fs_i + fd_j))*adj we compute
   G2 = exp(0.8*relu(s) + 0.2*fd_j)*adj  (= P * exp(-0.2*fs_i)),
   which normalizes to the same attention.  Two equivalent pipelines:
     ACT-form: t1 = Relu(fsb + fd_j), t2 = Exp(0.8*t1 + 0.2*fd_j), mask
     DVE-form: u = A8b * B_j (ts), w = max(u, D_j) (ts), mask
   with A8b = exp(0.8*fs_i) broadcast, B = exp(fd), D = exp(0.2*fd).
 - reciprocals are computed as exp(-ln(x)) on the scalar engine (the
   DVE RECIPROCAL instruction costs ~5.3us regardless of size).
 - elu(x) = max(x,0) - 1 + exp(min(x,0)).
"""

import sys

if "/opt/trn_rl_repo" not in sys.path:
    sys.path.insert(0, "/opt/trn_rl_repo")

import numpy as np
import ml_dtypes

BF = ml_dtypes.bfloat16

# problem dims (hardcoded per spec)
N, F, H, D, C = 4096, 256, 8, 64, 64
CORES = 8
SLOPE = 0.2

import os as _os

# engine-split tunables: #ACT-form tiles (of 32) and #mask ops on gpsimd
ACT1_N = int(_os.environ.get("K_ACT1", "15"))
GPS1_N = int(_os.environ.get("K_GPS1", "16"))
ACT2_N = int(_os.environ.get("K_ACT2", "10"))
GPS2_N = int(_os.environ.get("K_GPS2", "12"))
BUFS = int(_os.environ.get("K_BUFS", "3"))
K_DEBUG = int(_os.environ.get("K_DEBUG", "0"))

_BASS_CACHE = {}


def _spread(k, nt):
    """k tile indices spread evenly over range(nt) (Bresenham)."""
    return {t for t in range(nt) if ((t + 1) * k) // nt > (t * k) // nt}


def _emit(nc, tc, n, shard):
    """Emit the SPMD per-core graph. n = graph size (4096 full), shard = n//8."""
    import concourse.bass as bass
    import concourse.mybir as mybir
    from concourse.masks import make_identity

    bf = mybir.dt.bfloat16
    f32 = mybir.dt.float32
    AF = mybir.ActivationFunctionType
    OP = mybir.AluOpType
    NT = n // 128          # number of 128-row tiles
    NH = n // 2            # split-layout free width
    RG = [list(range(CORES))]
    C1 = C + 1             # who payload row: C cols + ones col

    # ---- dram I/O ----
    xT_d = nc.dram_tensor("xT", [F, n], bf, kind="ExternalInput")
    adjT_d = nc.dram_tensor("adjT", [n, n], bf, kind="ExternalInput")
    adjs_d = nc.dram_tensor("adjs", [n, shard], bf, kind="ExternalInput")
    wh_d = nc.dram_tensor("wh", [F, D], bf, kind="ExternalInput")
    whT_d = nc.dram_tensor("whT", [D, F], bf, kind="ExternalInput")
    a2_d = nc.dram_tensor("a2", [D, 2], bf, kind="ExternalInput")
    wo_d = nc.dram_tensor("wo", [D, C], bf, kind="ExternalInput")
    woT_d = nc.dram_tensor("woT", [C, D], bf, kind="ExternalInput")
    ao2_d = nc.dram_tensor("ao2", [C, 2], bf, kind="ExternalInput")
    rsel_d = nc.dram_tensor("rsel", [CORES, 1], bf, kind="ExternalInput")
    out_d = nc.dram_tensor("out", [shard, C], f32, kind="ExternalOutput")

    # collective bounce buffers: flat payload = [n, C1] who rows + [2, n] gT
    FLAT = n * C1 + 2 * n
    rs_in = nc.dram_tensor("rs_in", [1, FLAT], bf)
    ag_out = nc.dram_tensor("ag_out", [1, FLAT], bf, addr_space="Shared")
    who_w = rs_in.ap()[0:1, 0:n * C1].rearrange("one (r c) -> (one r) c", c=C1)
    g_w = rs_in.ap()[0:1, n * C1:FLAT].rearrange("one (g i) -> (one g) i", i=n)
    who_r = ag_out.ap()[0:1, 0:n * C1].rearrange("one (r c) -> (one r) c", c=C1)
    g_r = ag_out.ap()[0:1, n * C1:FLAT].rearrange("one (g i) -> (one g) i", i=n)

    from contextlib import ExitStack

    es = ExitStack()
    pers = es.enter_context(tc.tile_pool(name="pers", bufs=1))
    ppool = es.enter_context(tc.tile_pool(name="psum", bufs=1, space="PSUM"))
    pbig = ppool.tile([128, 4096], f32, name="pbig")

    # ---- prologue: weights ----
    xtp = tc.tile_pool(name="xtp", bufs=1)
    xtpool = xtp.__enter__()
    xt = []
    for k in range(2):
        t = xtpool.tile([128, n], bf, name=f"xt{k}")
        nc.sync.dma_start(t[:], xT_d[k * 128:(k + 1) * 128, :])
        xt.append(t)
    wf = []
    for k in range(2):
        t = pers.tile([128, D + 2], bf, name=f"wf{k}")
        nc.sync.dma_start(t[:, 0:D], wh_d[k * 128:(k + 1) * 128, :])
        wf.append(t)
    whTt = pers.tile([128, F], bf, name="whTt")
    nc.gpsimd.memset(whTt[:], 0.0)
    nc.sync.dma_start(whTt[0:D, :], whT_d[:])
    a2t = pers.tile([128, 2], bf, name="a2t")
    nc.gpsimd.memset(a2t[:], 0.0)
    nc.sync.dma_start(a2t[0:D, :], a2_d[:])
    rselt = pers.tile([CORES, 1], bf, name="rselt")
    nc.sync.dma_start(rselt[:], rsel_d[:])

    # waug = W_h @ a2 : [F, 2] (two 128-row tiles)
    for k in range(2):
        pw = pbig[0:128, k * 512:k * 512 + 2]
        nc.tensor.matmul(pw, lhsT=whTt[:, k * 128:(k + 1) * 128], rhs=a2t[:],
                         start=True, stop=True)
        nc.vector.tensor_copy(wf[k][:, D:D + 2], pw)

    # fsrc row via matmul: fsrc = x @ wsrc -> psum rows, split on partitions 0/32
    n_cc = n // 512
    cpg = max(1, NH // 512)  # 512-chunks per half
    for cc in range(n_cc):
        part = (cc // cpg) * 32  # matmul out base partition must be 0/32/64
        foff = 2048 + 512 * (cc % cpg)
        pr = pbig[part:part + 1, foff:foff + 512]
        for k in range(2):
            nc.tensor.matmul(pr, lhsT=wf[k][:, D:D + 1],
                             rhs=xt[k][:, cc * 512:(cc + 1) * 512],
                             start=(k == 0), stop=(k == 1))
    # copy psum fsrc rows -> sbuf (partition-aligned; rows 0 and 32)
    fr = xtpool.tile([33, NH], f32, name="fr")
    nc.vector.tensor_copy(fr[0:1, :], pbig[0:1, 2048:2048 + NH])
    nc.scalar.activation(fr[32:33, :], pbig[32:33, 2048:2048 + NH], AF.Copy)

    # Whaug = x @ [W | wsrc | wdst] -> per i-tile [128, D+2]
    whl = []
    fsd = pers.tile([128, 2 * NT], f32, name="fsd")
    for it in range(NT):
        pwh = pbig[0:128, (it % 4) * 512:(it % 4) * 512 + D + 2]
        for k in range(2):
            nc.tensor.matmul(pwh, lhsT=xt[k][:, it * 128:(it + 1) * 128],
                             rhs=wf[k][:], start=(k == 0), stop=(k == 1))
        t = pers.tile([128, D + 1], bf, name=f"whl{it}")
        nc.vector.tensor_copy(t[:, 0:D], pwh[:, 0:D])
        nc.gpsimd.memset(t[:, D:D + 1], 1.0)
        nc.vector.tensor_copy(fsd[:, 2 * it:2 * it + 2], pwh[:, D:D + 2])
        whl.append(t)

    # broadcast fsrc to all partitions; A8b = exp(0.8*fs_i).
    # NB: partition_broadcast on HW only reads from partition 0, so the
    # offset source row is first DMA-shifted to partition 0.
    frb = xtpool.tile([33, NH], bf, name="frb")
    nc.vector.tensor_copy(frb[0:1, :], fr[0:1, :])
    nc.vector.tensor_copy(frb[32:33, :], fr[32:33, :])
    frb2 = xtpool.tile([1, NH], bf, name="frb2")
    nc.sync.dma_start(frb2[0:1, :], frb[32:33, :])
    fsb = pers.tile([128, n], bf, name="fsb")
    nc.gpsimd.partition_broadcast(fsb[:, 0:NH], frb[0:1, :])
    nc.gpsimd.partition_broadcast(fsb[:, NH:n], frb2[0:1, :])
    A8b = pers.tile([128, n], bf, name="A8b")
    nc.scalar.activation(A8b[:], fsb[:], AF.Exp, scale=0.8)
    xtp.__exit__(None, None, None)

    # per-partition fd constants: raw fd, 0.2*fd, exp(fd), exp(0.2*fd)
    fsdr = fsd.rearrange("p (t two) -> p t two", two=2)
    fdc = pers.tile([128, NT], f32, name="fdc")
    fd02 = pers.tile([128, NT], f32, name="fd02")
    Bc = pers.tile([128, NT], f32, name="Bc")
    Dc = pers.tile([128, NT], f32, name="Dc")
    fdcr = fdc.rearrange("p (t o) -> p t o", o=1)
    fd02r = fd02.rearrange("p (t o) -> p t o", o=1)
    Bcr = Bc.rearrange("p (t o) -> p t o", o=1)
    Dcr = Dc.rearrange("p (t o) -> p t o", o=1)
    nc.vector.tensor_copy(fdcr[:], fsdr[:, :, 1:2])
    nc.vector.tensor_scalar(out=fd02r[:], in0=fsdr[:, :, 1:2], scalar1=SLOPE,
                            scalar2=None, op0=OP.mult)
    nc.scalar.activation(Bcr[:], fsdr[:, :, 1:2], AF.Exp)
    nc.scalar.activation(Dcr[:], fsdr[:, :, 1:2], AF.Exp, scale=SLOPE)

    # woaug = [Wo_h | Wo_h@ao_src | Wo_h@ao_dst]  [D, C+2], duplicated on
    # partitions 0:64 and 64:128 (matmul requires lhsT/rhs base partitions
    # to match; eluO halves live at 0 and 64)
    woTt = pers.tile([128, D], bf, name="woTt")
    nc.gpsimd.memset(woTt[:], 0.0)
    nc.sync.dma_start(woTt[0:C, :], woT_d[:])
    ao2t = pers.tile([128, 2], bf, name="ao2t")
    nc.gpsimd.memset(ao2t[:], 0.0)
    nc.sync.dma_start(ao2t[0:C, :], ao2_d[:])
    woaug = pers.tile([128, C + 2], bf, name="woaug")
    for half in range(2):
        pwo = pbig[half * 64:half * 64 + D, 0:2]
        nc.tensor.matmul(pwo, lhsT=woTt[:, 0:D], rhs=ao2t[:],
                         start=True, stop=True)
        nc.sync.dma_start(woaug[half * 64:half * 64 + D, 0:C], wo_d[:])
        nc.vector.tensor_copy(woaug[half * 64:half * 64 + D, C:C + 2], pwo)

    I128 = pers.tile([128, 128], f32, name="I128")
    make_identity(nc, I128[:])

    # ---- layer-1 j-loop ----
    l1es = ExitStack()
    adj_pool = l1es.enter_context(tc.tile_pool(name="adj", bufs=BUFS))
    t_pool = l1es.enter_context(tc.tile_pool(name="t1", bufs=BUFS))
    u_pool = l1es.enter_context(tc.tile_pool(name="t2", bufs=BUFS))
    p_pool = l1es.enter_context(tc.tile_pool(name="pp", bufs=BUFS))

    act_set = _spread(ACT1_N, NT)
    # masks to gpsimd: prefer ACT-form tiles (their chains avoid DVE)
    order = [t for t in range(NT) if t in act_set] + \
            [t for t in range(NT) if t not in act_set]
    gps_set = set(order[:GPS1_N])

    nchunk = n // 512
    for t in range(NT):
        adjt = adj_pool.tile([128, n], bf, tag="adjt")
        nc.sync.dma_start(adjt[:], adjT_d[t * 128:(t + 1) * 128, :])
        P = p_pool.tile([128, n], bf, tag="P")
        if t in act_set:
            tt1 = t_pool.tile([128, n], bf, tag="tt1")
            nc.scalar.activation(tt1[:], fsb[:], AF.Relu,
                                 bias=fdc[:, t:t + 1])
            uu = u_pool.tile([128, n], bf, tag="uu")
            nc.scalar.activation(uu[:], tt1[:], AF.Exp, scale=0.8,
                                 bias=fd02[:, t:t + 1])
        else:
            tt1 = t_pool.tile([128, n], bf, tag="tt1")
            nc.vector.tensor_scalar(out=tt1[:], in0=A8b[:],
                                    scalar1=Bc[:, t:t + 1],
                                    scalar2=None, op0=OP.mult)
            uu = u_pool.tile([128, n], bf, tag="uu")
            nc.vector.tensor_scalar(out=uu[:], in0=tt1[:],
                                    scalar1=Dc[:, t:t + 1],
                                    scalar2=None, op0=OP.max)
        if t in gps_set:
            nc.gpsimd.tensor_tensor(P[:], uu[:], adjt[:], OP.mult)
        else:
            nc.vector.tensor_tensor(P[:], uu[:], adjt[:], OP.mult)
        for c in range(nchunk):
            nc.tensor.matmul(pbig[0:D + 1, c * 512:(c + 1) * 512],
                             lhsT=whl[t][:], rhs=P[:, c * 512:(c + 1) * 512],
                             start=(t == 0), stop=(t == NT - 1))
    l1es.close()

    # ---- layer-1 epilogue: normalize + elu (split [128, NH] layout) ----
    # psum -> sbuf (partition-aligned compute copies on two engines);
    # transient tiles live in a scoped pool freed before layer-2 prep
    epp = tc.tile_pool(name="epp", bufs=1)
    ep = epp.__enter__()
    o1lo = ep.tile([D + 1, NH], f32, name="o1lo")
    o1hi = ep.tile([D + 1, NH], f32, name="o1hi")
    nc.vector.tensor_copy(o1lo[:], pbig[0:D + 1, 0:NH])
    nc.scalar.activation(o1hi[:], pbig[0:D + 1, NH:n], AF.Copy)
    # sbuf->sbuf DMAs to fold into a [128, NH] split layout
    o1s = ep.tile([128, NH], f32, name="o1s")
    nc.sync.dma_start(o1s[0:D, :], o1lo[0:D, :])
    nc.sync.dma_start(o1s[D:2 * D, :], o1hi[0:D, :])
    # 1/Z via exp(-ln(Z)) on the scalar engine (DVE reciprocal has a
    # ~5.3us fixed cost): shift Z rows to partition 0, invert, broadcast.
    # Lns then Exps batched to avoid activation-table swaps.
    zfa = ep.tile([1, NH], f32, name="zfa")
    zfb = ep.tile([1, NH], f32, name="zfb")
    zla = ep.tile([1, NH], bf, name="zla")
    zlb = ep.tile([1, NH], bf, name="zlb")
    zb = ep.tile([128, NH], bf, name="zb")
    zbx = ep.tile([D, NH], bf, name="zbx")
    nc.sync.dma_start(zfa[0:1, :], o1lo[D:D + 1, :])
    nc.sync.dma_start(zfb[0:1, :], o1hi[D:D + 1, :])
    nc.scalar.activation(zfa[0:1, :], zfa[0:1, :], AF.Ln)
    nc.scalar.activation(zfb[0:1, :], zfb[0:1, :], AF.Ln)
    nc.scalar.activation(zla[0:1, :], zfa[0:1, :], AF.Exp, scale=-1.0)
    nc.scalar.activation(zlb[0:1, :], zfb[0:1, :], AF.Exp, scale=-1.0)
    nc.gpsimd.partition_broadcast(zb[0:D, :], zla[0:1, :])
    nc.gpsimd.partition_broadcast(zbx[0:D, :], zlb[0:1, :])
    nc.sync.dma_start(zb[D:2 * D, :], zbx[0:D, :])
    o1n = ep.tile([128, NH], bf, name="o1n")
    nc.vector.tensor_tensor(o1n[:], o1s[:], zb[:], OP.mult)
    # elu
    mm = ep.tile([128, NH], bf, name="mm")
    nc.vector.tensor_scalar(out=mm[:], in0=o1n[:], scalar1=0.0, scalar2=None,
                            op0=OP.min)
    em = ep.tile([128, NH], bf, name="em")
    nc.scalar.activation(em[:], mm[:], AF.Exp)
    r1 = ep.tile([128, NH], bf, name="r1")
    nc.vector.tensor_scalar(out=r1[:], in0=o1n[:], scalar1=0.0, scalar2=-1.0,
                            op0=OP.max, op1=OP.add)
    eluO = pers.tile([128, NH], bf, name="eluO")
    nc.vector.tensor_tensor(eluO[:], r1[:], em[:], OP.add)

    # partial Who = eluO^T.T @ wo -> [n, C] into gts (col C holds 1/8 so the
    # AllReduce sum yields the ones column used for Z2); DMA to who region
    gts = pers.tile([128, NT, C1], bf, name="gts")
    nc.gpsimd.memset(gts[:, :, C:C + 1], 1.0 / CORES)
    half_t = NT // 2
    for it in range(NT):
        prt = (it // half_t) * D
        col = (it % half_t) * 128
        pt2 = pbig[0:128, (it % 8) * 512:(it % 8) * 512 + C]
        nc.tensor.matmul(pt2, lhsT=eluO[prt:prt + D, col:col + 128],
                         rhs=woaug[prt:prt + D, 0:C], start=True, stop=True)
        nc.vector.tensor_copy(gts[:, it, 0:C], pt2)
        nc.sync.dma_start(who_w[it * 128:(it + 1) * 128, :], gts[:, it, :])

    # gT = [g_src | g_dst]^T as [2, n]: out[r, i] = sum_d ao2[d, r]*eluO[d, i]
    for cc in range(n_cc):
        half = cc // cpg
        col = (cc % cpg) * 512
        pg = pbig[0:2, cc * 512:(cc + 1) * 512]
        nc.tensor.matmul(pg, lhsT=woaug[half * 64:half * 64 + D, C:C + 2],
                         rhs=eluO[half * 64:half * 64 + D, col:col + 512],
                         start=True, stop=True)
    gtt = ep.tile([2, n], bf, name="gtt")
    nc.vector.tensor_copy(gtt[:, 0:NH], pbig[0:2, 0:NH])
    nc.scalar.activation(gtt[:, NH:n], pbig[0:2, NH:n], AF.Copy)
    nc.sync.dma_start(g_w[:], gtt[:])
    epp.__exit__(None, None, None)

    # prefetch layer-2 adj slice (overlaps the collective)
    adjs2 = pers.tile([128, NT * shard], bf, name="adjs2")
    adjs2v = adjs2.rearrange("p (t i) -> p t i", t=NT)
    nc.scalar.dma_start(
        adjs2v[:], adjs_d.ap().rearrange("(t p) i -> p t i", p=128))

    # ---- collective: single AllReduce over the flat payload ----
    nc.gpsimd.collective_compute(
        "AllReduce", mybir.AluOpType.add, replica_groups=RG,
        ins=[rs_in.ap().rearrange("one (a b) -> (one a) b", a=CORES)],
        outs=[ag_out.ap().rearrange("one (a b) -> (one a) b", a=CORES)])

    # ---- layer-2 prep ----
    whol = pers.tile([128, NT, C1], bf, name="whol")
    for t in range(NT):
        nc.sync.dma_start(whol[:, t, :], who_r[t * 128:(t + 1) * 128, :])
    # g_dst -> [128, NT] via partition-split DMA of the flat row
    gdc = pers.tile([128, NT], bf, name="gdc")
    nc.sync.dma_start(
        gdc[:], g_r[1:2, :].rearrange("one (t p) -> (one p) t", p=128))
    gdf = pers.tile([128, NT], f32, name="gdf")
    gd02 = pers.tile([128, NT], f32, name="gd02")
    B2c = pers.tile([128, NT], f32, name="B2c")
    D2c = pers.tile([128, NT], f32, name="D2c")
    nc.vector.tensor_copy(gdf[:], gdc[:])
    nc.vector.tensor_scalar(out=gd02[:], in0=gdc[:], scalar1=SLOPE,
                            scalar2=None, op0=OP.mult)
    nc.scalar.activation(B2c[:], gdc[:], AF.Exp)
    nc.scalar.activation(D2c[:], gdc[:], AF.Exp, scale=SLOPE)
    # g_src slice for this core: one-hot rsel @ g_src viewed as [8, shard]
    gs8 = pers.tile([CORES, shard], bf, name="gs8")
    nc.sync.dma_start(
        gs8[:], g_r[0:1, :].rearrange("one (a i) -> (one a) i", a=CORES))
    pgs = pbig[0:1, 3584:3584 + shard]
    nc.tensor.matmul(pgs, lhsT=rselt[:], rhs=gs8[:], start=True, stop=True)
    gsr = pers.tile([1, shard], bf, name="gsr")
    nc.vector.tensor_copy(gsr[:], pgs)
    gsb = pers.tile([128, shard], bf, name="gsb")
    nc.gpsimd.partition_broadcast(gsb[:], gsr[0:1, :])
    A2b = pers.tile([128, shard], bf, name="A2b")
    nc.scalar.activation(A2b[:], gsb[:], AF.Exp, scale=0.8)

    # ---- layer-2 j-loop ----
    t_pool = es.enter_context(tc.tile_pool(name="t1b", bufs=BUFS))
    u_pool = es.enter_context(tc.tile_pool(name="t2b", bufs=BUFS))
    p_pool = es.enter_context(tc.tile_pool(name="ppb", bufs=BUFS))
    act2_set = _spread(ACT2_N, NT)
    order2 = [t for t in range(NT) if t in act2_set] + \
             [t for t in range(NT) if t not in act2_set]
    gps2_set = set(order2[:GPS2_N])
    for t in range(NT):
        adjs = adjs2[:, t * shard:(t + 1) * shard]
        P2 = p_pool.tile([128, shard], bf, tag="P2")
        if t in act2_set:
            q1 = t_pool.tile([128, shard], bf, tag="q1")
            nc.scalar.activation(q1[:], gsb[:], AF.Relu,
                                 bias=gdf[:, t:t + 1])
            q2 = u_pool.tile([128, shard], bf, tag="q2")
            nc.scalar.activation(q2[:], q1[:], AF.Exp, scale=0.8,
                                 bias=gd02[:, t:t + 1])
        else:
            q1 = t_pool.tile([128, shard], bf, tag="q1")
            nc.vector.tensor_scalar(out=q1[:], in0=A2b[:],
                                    scalar1=B2c[:, t:t + 1],
                                    scalar2=None, op0=OP.mult)
            q2 = u_pool.tile([128, shard], bf, tag="q2")
            nc.vector.tensor_scalar(out=q2[:], in0=q1[:],
                                    scalar1=D2c[:, t:t + 1],
                                    scalar2=None, op0=OP.max)
        if t in gps2_set:
            nc.gpsimd.tensor_tensor(P2[:], q2[:], adjs, OP.mult)
        else:
            nc.vector.tensor_tensor(P2[:], q2[:], adjs, OP.mult)
        nc.tensor.matmul(pbig[0:C + 1, 0:shard], lhsT=whol[:, t, :], rhs=P2[:],
                         start=(t == 0), stop=(t == NT - 1))

    if K_DEBUG:
        tap_fsd = nc.dram_tensor("tap_fsd", [128, 2 * NT], f32, kind="ExternalOutput")
        nc.sync.dma_start(tap_fsd.ap(), fsd[:])
        tap_o1lo = nc.dram_tensor("tap_o1lo", [D + 1, NH], f32, kind="ExternalOutput")
        nc.sync.dma_start(tap_o1lo.ap(), o1lo[:])
        tap_eluO = nc.dram_tensor("tap_eluO", [128, NH], bf, kind="ExternalOutput")
        nc.sync.dma_start(tap_eluO.ap(), eluO[:])
        tap_rsin = nc.dram_tensor("tap_rsin", [1, FLAT], bf, kind="ExternalOutput")
        nc.sync.dma_start(tap_rsin.ap(), rs_in.ap())
        tap_ag = nc.dram_tensor("tap_ag", [1, FLAT], bf, kind="ExternalOutput")
        nc.sync.dma_start(tap_ag.ap(), ag_out.ap())
        tap_gsb = nc.dram_tensor("tap_gsb", [128, shard], bf, kind="ExternalOutput")
        nc.sync.dma_start(tap_gsb.ap(), gsb[:])
        tap_gdf = nc.dram_tensor("tap_gdf", [128, NT], f32, kind="ExternalOutput")
        nc.sync.dma_start(tap_gdf.ap(), gdf[:])

    # ---- layer-2 epilogue: transpose, normalize, elu, log_softmax ----
    o2t = pers.tile([C + 1, shard], f32, name="o2t")
    nc.vector.tensor_copy(o2t[:], pbig[0:C + 1, 0:shard])
    if K_DEBUG:
        tap_o2t = nc.dram_tensor("tap_o2t", [C + 1, shard], f32, kind="ExternalOutput")
        nc.sync.dma_start(tap_o2t.ap(), o2t[:])
    # stage-major (all chunks per stage) so same-table ACT ops batch and
    # the engines pipeline across chunks
    nst = (shard + 127) // 128
    ws = [min(128, shard - k * 128) for k in range(nst)]
    ptrs = [pbig[0:ws[k], 512 + k * 512:512 + k * 512 + C + 1]
            for k in range(nst)]
    Tl = lambda nm, c=C: [pers.tile([128, c], f32, name=f"{nm}{k}")
                          for k in range(nst)]
    zr, o2n, m2, e2, r2, el2 = (Tl("zr", 1), Tl("o2n"), Tl("m2"), Tl("e2"),
                                Tl("r2"), Tl("el2"))
    mx, xm, ex, sume, lns, ok = (Tl("mx", 1), Tl("xm"), Tl("ex"),
                                 Tl("sume", 1), Tl("lns", 1), Tl("ok"))
    for k in range(nst):
        nc.tensor.transpose(ptrs[k], o2t[:, k * 128:k * 128 + ws[k]],
                            I128[0:C + 1, 0:C + 1])
    for k in range(nst):
        nc.scalar.activation(zr[k][0:ws[k], :], ptrs[k][:, C:C + 1], AF.Ln)
    for k in range(nst):
        nc.scalar.activation(zr[k][0:ws[k], :], zr[k][0:ws[k], :], AF.Exp,
                             scale=-1.0)
    for k in range(nst):
        w = ws[k]
        nc.vector.tensor_scalar(out=o2n[k][0:w, :], in0=ptrs[k][:, 0:C],
                                scalar1=zr[k][0:w, :], scalar2=None,
                                op0=OP.mult)
        nc.vector.tensor_scalar(out=m2[k][0:w, :], in0=o2n[k][0:w, :],
                                scalar1=0.0, scalar2=None, op0=OP.min)
    for k in range(nst):
        nc.scalar.activation(e2[k][0:ws[k], :], m2[k][0:ws[k], :], AF.Exp)
    for k in range(nst):
        w = ws[k]
        nc.vector.tensor_scalar(out=r2[k][0:w, :], in0=o2n[k][0:w, :],
                                scalar1=0.0, scalar2=-1.0, op0=OP.max,
                                op1=OP.add)
        nc.vector.tensor_tensor(el2[k][0:w, :], r2[k][0:w, :], e2[k][0:w, :],
                                OP.add)
        nc.vector.tensor_reduce(mx[k][0:w, :], el2[k][0:w, :],
                                mybir.AxisListType.X, OP.max)
        nc.vector.tensor_scalar(out=xm[k][0:w, :], in0=el2[k][0:w, :],
                                scalar1=mx[k][0:w, :], scalar2=None,
                                op0=OP.subtract)
    for k in range(nst):
        nc.scalar.activation(ex[k][0:ws[k], :], xm[k][0:ws[k], :], AF.Exp,
                             accum_out=sume[k][0:ws[k], :])
    for k in range(nst):
        nc.scalar.activation(lns[k][0:ws[k], :], sume[k][0:ws[k], :], AF.Ln)
    for k in range(nst):
        w = ws[k]
        nc.vector.tensor_scalar(out=ok[k][0:w, :], in0=xm[k][0:w, :],
                                scalar1=lns[k][0:w, :], scalar2=None,
                                op0=OP.subtract)
        nc.sync.dma_start(out_d[k * 128:k * 128 + w, :], ok[k][0:w, :])

    es.close()


def build(n=N, debug=False):
    from concourse import bacc
    import concourse.tile as tile

    nc = bacc.Bacc("TRN2", target_bir_lowering=False, debug=debug,
                   num_devices=CORES)
    with tile.TileContext(nc) as tc:
        _emit(nc, tc, n, n // CORES)
    nc.compile()
    return nc


def make_in_maps(x, adj, W, a, Wo, ao, n=N):
    """Host-side shard/layout prep -> list of 8 input dicts."""
    shard = n // CORES
    xT = np.ascontiguousarray(x.T).astype(BF)
    adjT = np.ascontiguousarray(adj.T).astype(BF)
    in_maps = []
    for h in range(CORES):
        wh = W[h].astype(BF)
        woh = Wo[h * D:(h + 1) * D, :].astype(BF)
        rsel = np.zeros((CORES, 1), dtype=BF)
        rsel[h, 0] = 1.0
        in_maps.append({
            "xT": xT,
            "adjT": adjT,
            "adjs": np.ascontiguousarray(adjT[:, h * shard:(h + 1) * shard]),
            "wh": wh,
            "whT": np.ascontiguousarray(wh.T),
            "a2": np.ascontiguousarray(np.stack([a[h, :D], a[h, D:]], axis=1)).astype(BF),
            "wo": woh,
            "woT": np.ascontiguousarray(woh.T),
            "ao2": np.ascontiguousarray(np.stack([ao[:C], ao[C:]], axis=1)).astype(BF),
            "rsel": rsel,
        })
    return in_maps


def kernel(x, adj, W, a, Wo, ao):
    from concourse.bass_utils import run_bass_kernel_spmd

    x = np.asarray(x, np.float32)
    adj = np.asarray(adj, np.float32)
    W = np.asarray(W, np.float32)
    a = np.asarray(a, np.float32)
    Wo = np.asarray(Wo, np.float32)
    ao = np.asarray(ao, np.float32)

    if "nc" not in _BASS_CACHE:
        _BASS_CACHE["nc"] = build()
    nc = _BASS_CACHE["nc"]
    in_maps = make_in_maps(x, adj, W, a, Wo, ao)
    r = run_bass_kernel_spmd(nc, in_maps, core_ids=list(range(CORES)))
    out = np.concatenate([r.results[c]["out"] for c in range(CORES)], axis=0)
    return np.asarray(out, np.float32)


# revision 19
# speedup vs baseline: 1.6545x; 1.0902x over previous
"""Trainium2 Bass kernel for a 2-layer GAT (nn_AGAEMD problem).

Sharding: layer-1 heads across 8 cores (core h owns head h, full N x N
attention for that head); layer-2 row-sharded (core c owns output rows
[c*512, (c+1)*512)).  Head outputs are combined with ONE bf16 AllReduce
over a flat contiguous payload (Who partials + a ones column + gT rows);
the per-core g_src slice is extracted post-AR with a one-hot selection
matmul (rsel input), avoiding any core-dependent addressing.

Math notes:
 - softmax rows are invariant to any per-column factor, so instead of
   P = exp(leaky(fs_i + fd_j))*adj we compute
   G2 = exp(0.8*relu(s) + 0.2*fd_j)*adj  (= P * exp(-0.2*fs_i)),
   which normalizes to the same attention.  Two equivalent pipelines:
     ACT-form: t1 = Relu(fsb + fd_j), t2 = Exp(0.8*t1 + 0.2*fd_j), mask
     DVE-form: u = A8b * B_j (ts), w = max(u, D_j) (ts), mask
   with A8b = exp(0.8*fs_i) broadcast, B = exp(fd), D = exp(0.2*fd).
 - reciprocals are computed as exp(-ln(x)) on the scalar engine (the
   DVE RECIPROCAL instruction costs ~5.3us regardless of size).
 - elu(x) = max(x,0) - 1 + exp(min(x,0)).
"""

import sys

if "/opt/trn_rl_repo" not in sys.path:
    sys.path.insert(0, "/opt/trn_rl_repo")

import numpy as np
import ml_dtypes

BF = ml_dtypes.bfloat16

# problem dims (hardcoded per spec)
N, F, H, D, C = 4096, 256, 8, 64, 64
CORES = 8
SLOPE = 0.2

import os as _os

# engine-split tunables: #ACT-form tiles (of 32) and #mask ops on gpsimd
ACT1_N = int(_os.environ.get("K_ACT1", "15"))
GPS1_N = int(_os.environ.get("K_GPS1", "16"))
ACT2_N = int(_os.environ.get("K_ACT2", "10"))
GPS2_N = int(_os.environ.get("K_GPS2", "12"))
BUFS = int(_os.environ.get("K_BUFS", "3"))
K_DEBUG = int(_os.environ.get("K_DEBUG", "0"))

_BASS_CACHE = {}


def _spread(k, nt):
    """k tile indices spread evenly over range(nt) (Bresenham)."""
    return {t for t in range(nt) if ((t + 1) * k) // nt > (t * k) // nt}


def _emit(nc, tc, n, shard):
    """Emit the SPMD per-core graph. n = graph size (4096 full), shard = n//8."""
    import concourse.bass as bass
    import concourse.mybir as mybir
    from concourse.masks import make_identity

    bf = mybir.dt.bfloat16
    f32 = mybir.dt.float32
    AF = mybir.ActivationFunctionType
    OP = mybir.AluOpType
    NT = n // 128          # number of 128-row tiles
    NH = n // 2            # split-layout free width
    RG = [list(range(CORES))]
    C1 = C + 1             # who payload row: C cols + ones col

    # ---- dram I/O ----
    xT_d = nc.dram_tensor("xT", [F, n], bf, kind="ExternalInput")
    adjT_d = nc.dram_tensor("adjT", [n, n], bf, kind="ExternalInput")
    adjs_d = nc.dram_tensor("adjs", [n, shard], bf, kind="ExternalInput")
    wh_d = nc.dram_tensor("wh", [F, D], bf, kind="ExternalInput")
    whT_d = nc.dram_tensor("whT", [D, F], bf, kind="ExternalInput")
    a2_d = nc.dram_tensor("a2", [D, 2], bf, kind="ExternalInput")
    wo_d = nc.dram_tensor("wo", [D, C], bf, kind="ExternalInput")
    woT_d = nc.dram_tensor("woT", [C, D], bf, kind="ExternalInput")
    ao2_d = nc.dram_tensor("ao2", [C, 2], bf, kind="ExternalInput")
    rsel_d = nc.dram_tensor("rsel", [CORES, 1], bf, kind="ExternalInput")
    out_d = nc.dram_tensor("out", [shard, C], f32, kind="ExternalOutput")

    # collective bounce buffers: flat payload = [n, C1] who rows + [2, n] gT
    FLAT = n * C1 + 2 * n
    rs_in = nc.dram_tensor("rs_in", [1, FLAT], bf)
    ag_out = nc.dram_tensor("ag_out", [1, FLAT], bf, addr_space="Shared")
    who_w = rs_in.ap()[0:1, 0:n * C1].rearrange("one (r c) -> (one r) c", c=C1)
    g_w = rs_in.ap()[0:1, n * C1:FLAT].rearrange("one (g i) -> (one g) i", i=n)
    who_r = ag_out.ap()[0:1, 0:n * C1].rearrange("one (r c) -> (one r) c", c=C1)
    g_r = ag_out.ap()[0:1, n * C1:FLAT].rearrange("one (g i) -> (one g) i", i=n)

    from contextlib import ExitStack

    es = ExitStack()
    pers = es.enter_context(tc.tile_pool(name="pers", bufs=1))
    ppool = es.enter_context(tc.tile_pool(name="psum", bufs=1, space="PSUM"))
    pbig = ppool.tile([128, 4096], f32, name="pbig")

    # ---- prologue: weights ----
    xtp = tc.tile_pool(name="xtp", bufs=1)
    xtpool = xtp.__enter__()
    xt = []
    for k in range(2):
        t = xtpool.tile([128, n], bf, name=f"xt{k}")
        nc.sync.dma_start(t[:], xT_d[k * 128:(k + 1) * 128, :])
        xt.append(t)
    wf = []
    for k in range(2):
        t = pers.tile([128, D + 2], bf, name=f"wf{k}")
        nc.sync.dma_start(t[:, 0:D], wh_d[k * 128:(k + 1) * 128, :])
        wf.append(t)
    whTt = pers.tile([128, F], bf, name="whTt")
    nc.gpsimd.memset(whTt[:], 0.0)
    nc.sync.dma_start(whTt[0:D, :], whT_d[:])
    a2t = pers.tile([128, 2], bf, name="a2t")
    nc.gpsimd.memset(a2t[:], 0.0)
    nc.sync.dma_start(a2t[0:D, :], a2_d[:])
    rselt = pers.tile([CORES, 1], bf, name="rselt")
    nc.sync.dma_start(rselt[:], rsel_d[:])

    # waug = W_h @ a2 : [F, 2] (two 128-row tiles)
    for k in range(2):
        pw = pbig[0:128, k * 512:k * 512 + 2]
        nc.tensor.matmul(pw, lhsT=whTt[:, k * 128:(k + 1) * 128], rhs=a2t[:],
                         start=True, stop=True)
        nc.vector.tensor_copy(wf[k][:, D:D + 2], pw)

    # fsrc row via matmul: fsrc = x @ wsrc -> psum rows, split on partitions 0/32
    n_cc = n // 512
    cpg = max(1, NH // 512)  # 512-chunks per half
    for cc in range(n_cc):
        part = (cc // cpg) * 32  # matmul out base partition must be 0/32/64
        foff = 2048 + 512 * (cc % cpg)
        pr = pbig[part:part + 1, foff:foff + 512]
        for k in range(2):
            nc.tensor.matmul(pr, lhsT=wf[k][:, D:D + 1],
                             rhs=xt[k][:, cc * 512:(cc + 1) * 512],
                             start=(k == 0), stop=(k == 1))
    # copy psum fsrc rows -> sbuf (partition-aligned; rows 0 and 32)
    fr = xtpool.tile([33, NH], f32, name="fr")
    nc.vector.tensor_copy(fr[0:1, :], pbig[0:1, 2048:2048 + NH])
    nc.scalar.activation(fr[32:33, :], pbig[32:33, 2048:2048 + NH], AF.Copy)

    # Whaug = x @ [W | wsrc | wdst] -> per i-tile [128, D+2]
    whl = []
    fsd = pers.tile([128, 2 * NT], f32, name="fsd")
    for it in range(NT):
        pwh = pbig[0:128, (it % 4) * 512:(it % 4) * 512 + D + 2]
        for k in range(2):
            nc.tensor.matmul(pwh, lhsT=xt[k][:, it * 128:(it + 1) * 128],
                             rhs=wf[k][:], start=(k == 0), stop=(k == 1))
        t = pers.tile([128, D + 1], bf, name=f"whl{it}")
        nc.vector.tensor_copy(t[:, 0:D], pwh[:, 0:D])
        nc.gpsimd.memset(t[:, D:D + 1], 1.0)
        nc.vector.tensor_copy(fsd[:, 2 * it:2 * it + 2], pwh[:, D:D + 2])
        whl.append(t)

    # broadcast fsrc to all partitions; A8b = exp(0.8*fs_i).
    # NB: partition_broadcast on HW only reads from partition 0, so the
    # offset source row is first DMA-shifted to partition 0.
    frb = xtpool.tile([33, NH], bf, name="frb")
    nc.vector.tensor_copy(frb[0:1, :], fr[0:1, :])
    nc.vector.tensor_copy(frb[32:33, :], fr[32:33, :])
    frb2 = xtpool.tile([1, NH], bf, name="frb2")
    nc.sync.dma_start(frb2[0:1, :], frb[32:33, :])
    fsb = pers.tile([128, n], bf, name="fsb")
    nc.gpsimd.partition_broadcast(fsb[:, 0:NH], frb[0:1, :])
    nc.gpsimd.partition_broadcast(fsb[:, NH:n], frb2[0:1, :])
    A8b = pers.tile([128, n], bf, name="A8b")
    nc.scalar.activation(A8b[:], fsb[:], AF.Exp, scale=0.8)
    xtp.__exit__(None, None, None)

    # per-partition fd constants: raw fd, 0.2*fd, exp(fd), exp(0.2*fd)
    fsdr = fsd.rearrange("p (t two) -> p t two", two=2)
    fdc = pers.tile([128, NT], f32, name="fdc")
    fd02 = pers.tile([128, NT], f32, name="fd02")
    Bc = pers.tile([128, NT], f32, name="Bc")
    Dc = pers.tile([128, NT], f32, name="Dc")
    fdcr = fdc.rearrange("p (t o) -> p t o", o=1)
    fd02r = fd02.rearrange("p (t o) -> p t o", o=1)
    Bcr = Bc.rearrange("p (t o) -> p t o", o=1)
    Dcr = Dc.rearrange("p (t o) -> p t o", o=1)
    nc.vector.tensor_copy(fdcr[:], fsdr[:, :, 1:2])
    nc.vector.tensor_scalar(out=fd02r[:], in0=fsdr[:, :, 1:2], scalar1=SLOPE,
                            scalar2=None, op0=OP.mult)
    nc.scalar.activation(Bcr[:], fsdr[:, :, 1:2], AF.Exp)
    nc.scalar.activation(Dcr[:], fsdr[:, :, 1:2], AF.Exp, scale=SLOPE)

    # woaug = [Wo_h | Wo_h@ao_src | Wo_h@ao_dst]  [D, C+2], duplicated on
    # partitions 0:64 and 64:128 (matmul requires lhsT/rhs base partitions
    # to match; eluO halves live at 0 and 64)
    woTt = pers.tile([128, D], bf, name="woTt")
    nc.gpsimd.memset(woTt[:], 0.0)
    nc.sync.dma_start(woTt[0:C, :], woT_d[:])
    ao2t = pers.tile([128, 2], bf, name="ao2t")
    nc.gpsimd.memset(ao2t[:], 0.0)
    nc.sync.dma_start(ao2t[0:C, :], ao2_d[:])
    woaug = pers.tile([128, C + 2], bf, name="woaug")
    for half in range(2):
        pwo = pbig[half * 64:half * 64 + D, 0:2]
        nc.tensor.matmul(pwo, lhsT=woTt[:, 0:D], rhs=ao2t[:],
                         start=True, stop=True)
        nc.sync.dma_start(woaug[half * 64:half * 64 + D, 0:C], wo_d[:])
        nc.vector.tensor_copy(woaug[half * 64:half * 64 + D, C:C + 2], pwo)

    I128 = pers.tile([128, 128], f32, name="I128")
    make_identity(nc, I128[:])

    # ---- layer-1 j-loop ----
    l1es = ExitStack()
    adj_pool = l1es.enter_context(tc.tile_pool(name="adj", bufs=BUFS))
    t_pool = l1es.enter_context(tc.tile_pool(name="t1", bufs=BUFS))
    u_pool = l1es.enter_context(tc.tile_pool(name="t2", bufs=BUFS))
    p_pool = l1es.enter_context(tc.tile_pool(name="pp", bufs=BUFS))

    act_set = _spread(ACT1_N, NT)
    # masks to gpsimd: prefer ACT-form tiles (their chains avoid DVE)
    order = [t for t in range(NT) if t in act_set] + \
            [t for t in range(NT) if t not in act_set]
    gps_set = set(order[:GPS1_N])

    # layer-2 adj slice, prefetched mid-loop (DMA has slack there) so the
    # transfer is done before the collective window
    adjs2 = pers.tile([128, NT * shard], bf, name="adjs2")
    adjs2v = adjs2.rearrange("p (t i) -> p t i", t=NT)

    nchunk = n // 512
    for t in range(NT):
        adjt = adj_pool.tile([128, n], bf, tag="adjt")
        eng = nc.sync if t % 2 == 0 else nc.scalar
        eng.dma_start(adjt[:], adjT_d[t * 128:(t + 1) * 128, :])
        if t == 20:
            nc.scalar.dma_start(
                adjs2v[:], adjs_d.ap().rearrange("(t p) i -> p t i", p=128))
        P = p_pool.tile([128, n], bf, tag="P")
        if t in act_set:
            tt1 = t_pool.tile([128, n], bf, tag="tt1")
            nc.scalar.activation(tt1[:], fsb[:], AF.Relu,
                                 bias=fdc[:, t:t + 1])
            uu = u_pool.tile([128, n], bf, tag="uu")
            nc.scalar.activation(uu[:], tt1[:], AF.Exp, scale=0.8,
                                 bias=fd02[:, t:t + 1])
        else:
            tt1 = t_pool.tile([128, n], bf, tag="tt1")
            nc.vector.tensor_scalar(out=tt1[:], in0=A8b[:],
                                    scalar1=Bc[:, t:t + 1],
                                    scalar2=None, op0=OP.mult)
            uu = u_pool.tile([128, n], bf, tag="uu")
            nc.vector.tensor_scalar(out=uu[:], in0=tt1[:],
                                    scalar1=Dc[:, t:t + 1],
                                    scalar2=None, op0=OP.max)
        if t in gps_set:
            nc.gpsimd.tensor_tensor(P[:], uu[:], adjt[:], OP.mult)
        else:
            nc.vector.tensor_tensor(P[:], uu[:], adjt[:], OP.mult)
        for c in range(nchunk):
            nc.tensor.matmul(pbig[0:D + 1, c * 512:(c + 1) * 512],
                             lhsT=whl[t][:], rhs=P[:, c * 512:(c + 1) * 512],
                             start=(t == 0), stop=(t == NT - 1))
    l1es.close()

    # ---- layer-1 epilogue: normalize + elu (split [128, NH] layout) ----
    # psum -> sbuf (partition-aligned compute copies on two engines);
    # transient tiles live in a scoped pool freed before layer-2 prep
    epp = tc.tile_pool(name="epp", bufs=1)
    ep = epp.__enter__()
    o1lo = ep.tile([D + 1, NH], f32, name="o1lo")
    o1hi = ep.tile([D + 1, NH], f32, name="o1hi")
    nc.vector.tensor_copy(o1lo[:], pbig[0:D + 1, 0:NH])
    nc.scalar.activation(o1hi[:], pbig[0:D + 1, NH:n], AF.Copy)
    # sbuf->sbuf DMAs to fold into a [128, NH] split layout
    o1s = ep.tile([128, NH], f32, name="o1s")
    nc.sync.dma_start(o1s[0:D, :], o1lo[0:D, :])
    nc.sync.dma_start(o1s[D:2 * D, :], o1hi[0:D, :])
    # 1/Z via exp(-ln(Z)) on the scalar engine (DVE reciprocal has a
    # ~5.3us fixed cost): shift Z rows to partition 0, invert, broadcast.
    # Lns then Exps batched to avoid activation-table swaps.
    zfa = ep.tile([1, NH], f32, name="zfa")
    zfb = ep.tile([1, NH], f32, name="zfb")
    zla = ep.tile([1, NH], bf, name="zla")
    zlb = ep.tile([1, NH], bf, name="zlb")
    zb = ep.tile([128, NH], bf, name="zb")
    zbx = ep.tile([D, NH], bf, name="zbx")
    nc.sync.dma_start(zfa[0:1, :], o1lo[D:D + 1, :])
    nc.sync.dma_start(zfb[0:1, :], o1hi[D:D + 1, :])
    nc.scalar.activation(zfa[0:1, :], zfa[0:1, :], AF.Ln)
    nc.scalar.activation(zfb[0:1, :], zfb[0:1, :], AF.Ln)
    nc.scalar.activation(zla[0:1, :], zfa[0:1, :], AF.Exp, scale=-1.0)
    nc.scalar.activation(zlb[0:1, :], zfb[0:1, :], AF.Exp, scale=-1.0)
    nc.gpsimd.partition_broadcast(zb[0:D, :], zla[0:1, :])
    nc.gpsimd.partition_broadcast(zbx[0:D, :], zlb[0:1, :])
    nc.sync.dma_start(zb[D:2 * D, :], zbx[0:D, :])
    o1n = ep.tile([128, NH], bf, name="o1n")
    nc.vector.tensor_tensor(o1n[:], o1s[:], zb[:], OP.mult)
    # elu
    mm = ep.tile([128, NH], bf, name="mm")
    nc.vector.tensor_scalar(out=mm[:], in0=o1n[:], scalar1=0.0, scalar2=None,
                            op0=OP.min)
    em = ep.tile([128, NH], bf, name="em")
    nc.scalar.activation(em[:], mm[:], AF.Exp)
    r1 = ep.tile([128, NH], bf, name="r1")
    nc.vector.tensor_scalar(out=r1[:], in0=o1n[:], scalar1=0.0, scalar2=-1.0,
                            op0=OP.max, op1=OP.add)
    eluO = pers.tile([128, NH], bf, name="eluO")
    nc.vector.tensor_tensor(eluO[:], r1[:], em[:], OP.add)

    # partial Who = eluO^T.T @ wo -> [n, C] into gts (col C holds 1/8 so the
    # AllReduce sum yields the ones column used for Z2); DMA to who region
    gts = pers.tile([128, NT, C1], bf, name="gts")
    nc.gpsimd.memset(gts[:, :, C:C + 1], 1.0 / CORES)
    half_t = NT // 2
    for it in range(NT):
        prt = (it // half_t) * D
        col = (it % half_t) * 128
        pt2 = pbig[0:128, (it % 8) * 512:(it % 8) * 512 + C]
        nc.tensor.matmul(pt2, lhsT=eluO[prt:prt + D, col:col + 128],
                         rhs=woaug[prt:prt + D, 0:C], start=True, stop=True)
        nc.vector.tensor_copy(gts[:, it, 0:C], pt2)
        nc.sync.dma_start(who_w[it * 128:(it + 1) * 128, :], gts[:, it, :])

    # gT = [g_src | g_dst]^T as [2, n]: out[r, i] = sum_d ao2[d, r]*eluO[d, i]
    for cc in range(n_cc):
        half = cc // cpg
        col = (cc % cpg) * 512
        pg = pbig[0:2, cc * 512:(cc + 1) * 512]
        nc.tensor.matmul(pg, lhsT=woaug[half * 64:half * 64 + D, C:C + 2],
                         rhs=eluO[half * 64:half * 64 + D, col:col + 512],
                         start=True, stop=True)
    gtt = ep.tile([2, n], bf, name="gtt")
    nc.vector.tensor_copy(gtt[:, 0:NH], pbig[0:2, 0:NH])
    nc.scalar.activation(gtt[:, NH:n], pbig[0:2, NH:n], AF.Copy)
    nc.sync.dma_start(g_w[:], gtt[:])
    epp.__exit__(None, None, None)

    # prefetch layer-2 adj slice (overlaps the collective)
    adjs2 = pers.tile([128, NT * shard], bf, name="adjs2")
    adjs2v = adjs2.rearrange("p (t i) -> p t i", t=NT)
    nc.scalar.dma_start(
        adjs2v[:], adjs_d.ap().rearrange("(t p) i -> p t i", p=128))

    # ---- collective: single AllReduce over the flat payload ----
    nc.gpsimd.collective_compute(
        "AllReduce", mybir.AluOpType.add, replica_groups=RG,
        ins=[rs_in.ap().rearrange("one (a b) -> (one a) b", a=CORES)],
        outs=[ag_out.ap().rearrange("one (a b) -> (one a) b", a=CORES)])

    # ---- layer-2 prep ----
    whol = pers.tile([128, NT, C1], bf, name="whol")
    for t in range(NT):
        nc.sync.dma_start(whol[:, t, :], who_r[t * 128:(t + 1) * 128, :])
    # g_dst -> [128, NT] via partition-split DMA of the flat row
    gdc = pers.tile([128, NT], bf, name="gdc")
    nc.sync.dma_start(
        gdc[:], g_r[1:2, :].rearrange("one (t p) -> (one p) t", p=128))
    gdf = pers.tile([128, NT], f32, name="gdf")
    gd02 = pers.tile([128, NT], f32, name="gd02")
    B2c = pers.tile([128, NT], f32, name="B2c")
    D2c = pers.tile([128, NT], f32, name="D2c")
    nc.vector.tensor_copy(gdf[:], gdc[:])
    nc.vector.tensor_scalar(out=gd02[:], in0=gdc[:], scalar1=SLOPE,
                            scalar2=None, op0=OP.mult)
    nc.scalar.activation(B2c[:], gdc[:], AF.Exp)
    nc.scalar.activation(D2c[:], gdc[:], AF.Exp, scale=SLOPE)
    # g_src slice for this core: one-hot rsel @ g_src viewed as [8, shard]
    gs8 = pers.tile([CORES, shard], bf, name="gs8")
    nc.sync.dma_start(
        gs8[:], g_r[0:1, :].rearrange("one (a i) -> (one a) i", a=CORES))
    pgs = pbig[0:1, 3584:3584 + shard]
    nc.tensor.matmul(pgs, lhsT=rselt[:], rhs=gs8[:], start=True, stop=True)
    gsr = pers.tile([1, shard], bf, name="gsr")
    nc.vector.tensor_copy(gsr[:], pgs)
    gsb = pers.tile([128, shard], bf, name="gsb")
    nc.gpsimd.partition_broadcast(gsb[:], gsr[0:1, :])
    A2b = pers.tile([128, shard], bf, name="A2b")
    nc.scalar.activation(A2b[:], gsb[:], AF.Exp, scale=0.8)

    # ---- layer-2 j-loop ----
    t_pool = es.enter_context(tc.tile_pool(name="t1b", bufs=BUFS))
    u_pool = es.enter_context(tc.tile_pool(name="t2b", bufs=BUFS))
    p_pool = es.enter_context(tc.tile_pool(name="ppb", bufs=BUFS))
    act2_set = _spread(ACT2_N, NT)
    order2 = [t for t in range(NT) if t in act2_set] + \
             [t for t in range(NT) if t not in act2_set]
    gps2_set = set(order2[:GPS2_N])
    for t in range(NT):
        adjs = adjs2[:, t * shard:(t + 1) * shard]
        P2 = p_pool.tile([128, shard], bf, tag="P2")
        if t in act2_set:
            q1 = t_pool.tile([128, shard], bf, tag="q1")
            nc.scalar.activation(q1[:], gsb[:], AF.Relu,
                                 bias=gdf[:, t:t + 1])
            q2 = u_pool.tile([128, shard], bf, tag="q2")
            nc.scalar.activation(q2[:], q1[:], AF.Exp, scale=0.8,
                                 bias=gd02[:, t:t + 1])
        else:
            q1 = t_pool.tile([128, shard], bf, tag="q1")
            nc.vector.tensor_scalar(out=q1[:], in0=A2b[:],
                                    scalar1=B2c[:, t:t + 1],
                                    scalar2=None, op0=OP.mult)
            q2 = u_pool.tile([128, shard], bf, tag="q2")
            nc.vector.tensor_scalar(out=q2[:], in0=q1[:],
                                    scalar1=D2c[:, t:t + 1],
                                    scalar2=None, op0=OP.max)
        if t in gps2_set:
            nc.gpsimd.tensor_tensor(P2[:], q2[:], adjs, OP.mult)
        else:
            nc.vector.tensor_tensor(P2[:], q2[:], adjs, OP.mult)
        nc.tensor.matmul(pbig[0:C + 1, 0:shard], lhsT=whol[:, t, :], rhs=P2[:],
                         start=(t == 0), stop=(t == NT - 1))

    if K_DEBUG:
        tap_fsd = nc.dram_tensor("tap_fsd", [128, 2 * NT], f32, kind="ExternalOutput")
        nc.sync.dma_start(tap_fsd.ap(), fsd[:])
        tap_o1lo = nc.dram_tensor("tap_o1lo", [D + 1, NH], f32, kind="ExternalOutput")
        nc.sync.dma_start(tap_o1lo.ap(), o1lo[:])
        tap_eluO = nc.dram_tensor("tap_eluO", [128, NH], bf, kind="ExternalOutput")
        nc.sync.dma_start(tap_eluO.ap(), eluO[:])
        tap_rsin = nc.dram_tensor("tap_rsin", [1, FLAT], bf, kind="ExternalOutput")
        nc.sync.dma_start(tap_rsin.ap(), rs_in.ap())
        tap_ag = nc.dram_tensor("tap_ag", [1, FLAT], bf, kind="ExternalOutput")
        nc.sync.dma_start(tap_ag.ap(), ag_out.ap())
        tap_gsb = nc.dram_tensor("tap_gsb", [128, shard], bf, kind="ExternalOutput")
        nc.sync.dma_start(tap_gsb.ap(), gsb[:])
        tap_gdf = nc.dram_tensor("tap_gdf", [128, NT], f32, kind="ExternalOutput")
        nc.sync.dma_start(tap_gdf.ap(), gdf[:])

    # ---- layer-2 epilogue: transpose, normalize, elu, log_softmax ----
    o2t = pers.tile([C + 1, shard], f32, name="o2t")
    nc.vector.tensor_copy(o2t[:], pbig[0:C + 1, 0:shard])
    if K_DEBUG:
        tap_o2t = nc.dram_tensor("tap_o2t", [C + 1, shard], f32, kind="ExternalOutput")
        nc.sync.dma_start(tap_o2t.ap(), o2t[:])
    # stage-major (all chunks per stage) so same-table ACT ops batch and
    # the engines pipeline across chunks
    nst = (shard + 127) // 128
    ws = [min(128, shard - k * 128) for k in range(nst)]
    ptrs = [pbig[0:ws[k], 512 + k * 512:512 + k * 512 + C + 1]
            for k in range(nst)]
    Tl = lambda nm, c=C: [pers.tile([128, c], f32, name=f"{nm}{k}")
                          for k in range(nst)]
    zr, o2n, m2, e2, r2, el2 = (Tl("zr", 1), Tl("o2n"), Tl("m2"), Tl("e2"),
                                Tl("r2"), Tl("el2"))
    mx, xm, ex, sume, lns, ok = (Tl("mx", 1), Tl("xm"), Tl("ex"),
                                 Tl("sume", 1), Tl("lns", 1), Tl("ok"))
    for k in range(nst):
        nc.tensor.transpose(ptrs[k], o2t[:, k * 128:k * 128 + ws[k]],
                            I128[0:C + 1, 0:C + 1])
    for k in range(nst):
        nc.scalar.activation(zr[k][0:ws[k], :], ptrs[k][:, C:C + 1], AF.Ln)
    for k in range(nst):
        nc.scalar.activation(zr[k][0:ws[k], :], zr[k][0:ws[k], :], AF.Exp,
                             scale=-1.0)
    for k in range(nst):
        w = ws[k]
        nc.vector.tensor_scalar(out=o2n[k][0:w, :], in0=ptrs[k][:, 0:C],
                                scalar1=zr[k][0:w, :], scalar2=None,
                                op0=OP.mult)
        nc.vector.tensor_scalar(out=m2[k][0:w, :], in0=o2n[k][0:w, :],
                                scalar1=0.0, scalar2=None, op0=OP.min)
    for k in range(nst):
        nc.scalar.activation(e2[k][0:ws[k], :], m2[k][0:ws[k], :], AF.Exp)
    for k in range(nst):
        w = ws[k]
        nc.vector.tensor_scalar(out=r2[k][0:w, :], in0=o2n[k][0:w, :],
                                scalar1=0.0, scalar2=-1.0, op0=OP.max,
                                op1=OP.add)
        nc.vector.tensor_tensor(el2[k][0:w, :], r2[k][0:w, :], e2[k][0:w, :],
                                OP.add)
        nc.vector.tensor_reduce(mx[k][0:w, :], el2[k][0:w, :],
                                mybir.AxisListType.X, OP.max)
        nc.vector.tensor_scalar(out=xm[k][0:w, :], in0=el2[k][0:w, :],
                                scalar1=mx[k][0:w, :], scalar2=None,
                                op0=OP.subtract)
    for k in range(nst):
        nc.scalar.activation(ex[k][0:ws[k], :], xm[k][0:ws[k], :], AF.Exp,
                             accum_out=sume[k][0:ws[k], :])
    for k in range(nst):
        nc.scalar.activation(lns[k][0:ws[k], :], sume[k][0:ws[k], :], AF.Ln)
    for k in range(nst):
        w = ws[k]
        nc.vector.tensor_scalar(out=ok[k][0:w, :], in0=xm[k][0:w, :],
                                scalar1=lns[k][0:w, :], scalar2=None,
                                op0=OP.subtract)
        nc.sync.dma_start(out_d[k * 128:k * 128 + w, :], ok[k][0:w, :])

    es.close()


def build(n=N, debug=False):
    from concourse import bacc
    import concourse.tile as tile

    nc = bacc.Bacc("TRN2", target_bir_lowering=False, debug=debug,
                   num_devices=CORES)
    with tile.TileContext(nc) as tc:
        _emit(nc, tc, n, n // CORES)
    nc.compile()
    return nc


def make_in_maps(x, adj, W, a, Wo, ao, n=N):
    """Host-side shard/layout prep -> list of 8 input dicts."""
    shard = n // CORES
    xT = np.ascontiguousarray(x.T).astype(BF)
    adjT = np.ascontiguousarray(adj.T).astype(BF)
    in_maps = []
    for h in range(CORES):
        wh = W[h].astype(BF)
        woh = Wo[h * D:(h + 1) * D, :].astype(BF)
        rsel = np.zeros((CORES, 1), dtype=BF)
        rsel[h, 0] = 1.0
        in_maps.append({
            "xT": xT,
            "adjT": adjT,
            "adjs": np.ascontiguousarray(adjT[:, h * shard:(h + 1) * shard]),
            "wh": wh,
            "whT": np.ascontiguousarray(wh.T),
            "a2": np.ascontiguousarray(np.stack([a[h, :D], a[h, D:]], axis=1)).astype(BF),
            "wo": woh,
            "woT": np.ascontiguousarray(woh.T),
            "ao2": np.ascontiguousarray(np.stack([ao[:C], ao[C:]], axis=1)).astype(BF),
            "rsel": rsel,
        })
    return in_maps


def kernel(x, adj, W, a, Wo, ao):
    from concourse.bass_utils import run_bass_kernel_spmd

    x = np.asarray(x, np.float32)
    adj = np.asarray(adj, np.float32)
    W = np.asarray(W, np.float32)
    a = np.asarray(a, np.float32)
    Wo = np.asarray(Wo, np.float32)
    ao = np.asarray(ao, np.float32)

    if "nc" not in _BASS_CACHE:
        _BASS_CACHE["nc"] = build()
    nc = _BASS_CACHE["nc"]
    in_maps = make_in_maps(x, adj, W, a, Wo, ao)
    r = run_bass_kernel_spmd(nc, in_maps, core_ids=list(range(CORES)))
    out = np.concatenate([r.results[c]["out"] for c in range(CORES)], axis=0)
    return np.asarray(out, np.float32)


# revision 22
# speedup vs baseline: 1.7885x; 1.0810x over previous
"""Trainium2 Bass kernel for a 2-layer GAT (nn_AGAEMD problem).

Sharding: layer-1 heads across 8 cores (core h owns head h, full N x N
attention for that head); layer-2 row-sharded (core c owns output rows
[c*512, (c+1)*512)).  Head outputs are combined with ONE bf16 AllReduce
over a flat contiguous payload (Who partials + a ones column + gT rows);
the per-core g_src slice is extracted post-AR with a one-hot selection
matmul (rsel input), avoiding any core-dependent addressing.

Math notes:
 - softmax rows are invariant to any per-column factor, so instead of
   P = exp(leaky(fs_i + fd_j))*adj we compute
   G2 = exp(0.8*relu(s) + 0.2*fd_j)*adj  (= P * exp(-0.2*fs_i)),
   which normalizes to the same attention.  Two equivalent pipelines:
     ACT-form: t1 = Relu(fsb + fd_j), t2 = Exp(0.8*t1 + 0.2*fd_j), mask
     DVE-form: u = A8b * B_j (ts), w = max(u, D_j) (ts), mask
   with A8b = exp(0.8*fs_i) broadcast, B = exp(fd), D = exp(0.2*fd).
 - reciprocals are computed as exp(-ln(x)) on the scalar engine (the
   DVE RECIPROCAL instruction costs ~5.3us regardless of size).
 - elu(x) = max(x,0) - 1 + exp(min(x,0)).
"""

import sys

if "/opt/trn_rl_repo" not in sys.path:
    sys.path.insert(0, "/opt/trn_rl_repo")

import numpy as np
import ml_dtypes

BF = ml_dtypes.bfloat16

# problem dims (hardcoded per spec)
N, F, H, D, C = 4096, 256, 8, 64, 64
CORES = 8
SLOPE = 0.2

import os as _os

# engine-split tunables: #ACT-form tiles (of 32) and #mask ops on gpsimd
ACT1_N = int(_os.environ.get("K_ACT1", "15"))
GPS1_N = int(_os.environ.get("K_GPS1", "16"))
ACT2_N = int(_os.environ.get("K_ACT2", "10"))
GPS2_N = int(_os.environ.get("K_GPS2", "12"))
BUFS = int(_os.environ.get("K_BUFS", "3"))
K_DEBUG = int(_os.environ.get("K_DEBUG", "0"))

_BASS_CACHE = {}


def _spread(k, nt):
    """k tile indices spread evenly over range(nt) (Bresenham)."""
    return {t for t in range(nt) if ((t + 1) * k) // nt > (t * k) // nt}


def _emit(nc, tc, n, shard):
    """Emit the SPMD per-core graph. n = graph size (4096 full), shard = n//8."""
    import concourse.bass as bass
    import concourse.mybir as mybir
    from concourse.masks import make_identity

    bf = mybir.dt.bfloat16
    f32 = mybir.dt.float32
    AF = mybir.ActivationFunctionType
    OP = mybir.AluOpType
    NT = n // 128          # number of 128-row tiles
    NH = n // 2            # split-layout free width
    RG = [list(range(CORES))]
    C1 = C + 1             # who payload row: C cols + ones col

    # ---- dram I/O ----
    xT_d = nc.dram_tensor("xT", [F, n], bf, kind="ExternalInput")
    adjT_d = nc.dram_tensor("adjT", [n, n], bf, kind="ExternalInput")
    adjs_d = nc.dram_tensor("adjs", [n, shard], bf, kind="ExternalInput")
    wh_d = nc.dram_tensor("wh", [F, D], bf, kind="ExternalInput")
    whT_d = nc.dram_tensor("whT", [D, F], bf, kind="ExternalInput")
    a2_d = nc.dram_tensor("a2", [D, 2], bf, kind="ExternalInput")
    wo_d = nc.dram_tensor("wo", [D, C], bf, kind="ExternalInput")
    woT_d = nc.dram_tensor("woT", [C, D], bf, kind="ExternalInput")
    ao2_d = nc.dram_tensor("ao2", [C, 2], bf, kind="ExternalInput")
    rsel_d = nc.dram_tensor("rsel", [CORES, 1], bf, kind="ExternalInput")
    out_d = nc.dram_tensor("out", [shard, C], f32, kind="ExternalOutput")

    # collective bounce buffers: flat payload = [n, C1] who rows + [2, n] gT
    FLAT = n * C1 + 2 * n
    rs_in = nc.dram_tensor("rs_in", [1, FLAT], bf)
    ag_out = nc.dram_tensor("ag_out", [1, FLAT], bf, addr_space="Shared")
    who_w = rs_in.ap()[0:1, 0:n * C1].rearrange("one (r c) -> (one r) c", c=C1)
    g_w = rs_in.ap()[0:1, n * C1:FLAT].rearrange("one (g i) -> (one g) i", i=n)
    who_r = ag_out.ap()[0:1, 0:n * C1].rearrange("one (r c) -> (one r) c", c=C1)
    g_r = ag_out.ap()[0:1, n * C1:FLAT].rearrange("one (g i) -> (one g) i", i=n)

    from contextlib import ExitStack

    es = ExitStack()
    pers = es.enter_context(tc.tile_pool(name="pers", bufs=1))
    ppool = es.enter_context(tc.tile_pool(name="psum", bufs=1, space="PSUM"))
    pbig = ppool.tile([128, 4096], f32, name="pbig")

    # ---- prologue: weights ----
    xtp = tc.tile_pool(name="xtp", bufs=1)
    xtpool = xtp.__enter__()
    xt = []
    for k in range(2):
        t = xtpool.tile([128, n], bf, name=f"xt{k}")
        nc.sync.dma_start(t[:], xT_d[k * 128:(k + 1) * 128, :])
        xt.append(t)
    wf = []
    for k in range(2):
        t = pers.tile([128, D + 2], bf, name=f"wf{k}")
        nc.sync.dma_start(t[:, 0:D], wh_d[k * 128:(k + 1) * 128, :])
        wf.append(t)
    whTt = pers.tile([128, F], bf, name="whTt")
    nc.gpsimd.memset(whTt[:], 0.0)
    nc.sync.dma_start(whTt[0:D, :], whT_d[:])
    a2t = pers.tile([128, 2], bf, name="a2t")
    nc.gpsimd.memset(a2t[:], 0.0)
    nc.sync.dma_start(a2t[0:D, :], a2_d[:])
    rselt = pers.tile([CORES, 1], bf, name="rselt")
    nc.sync.dma_start(rselt[:], rsel_d[:])

    # waug = W_h @ a2 : [F, 2] (two 128-row tiles)
    for k in range(2):
        pw = pbig[0:128, k * 512:k * 512 + 2]
        nc.tensor.matmul(pw, lhsT=whTt[:, k * 128:(k + 1) * 128], rhs=a2t[:],
                         start=True, stop=True)
        nc.vector.tensor_copy(wf[k][:, D:D + 2], pw)

    # fsrc row via matmul: fsrc = x @ wsrc -> psum rows, split on partitions 0/32
    n_cc = n // 512
    cpg = max(1, NH // 512)  # 512-chunks per half
    for cc in range(n_cc):
        part = (cc // cpg) * 32  # matmul out base partition must be 0/32/64
        foff = 2048 + 512 * (cc % cpg)
        pr = pbig[part:part + 1, foff:foff + 512]
        for k in range(2):
            nc.tensor.matmul(pr, lhsT=wf[k][:, D:D + 1],
                             rhs=xt[k][:, cc * 512:(cc + 1) * 512],
                             start=(k == 0), stop=(k == 1))
    # copy psum fsrc rows -> sbuf (partition-aligned; rows 0 and 32)
    fr = xtpool.tile([33, NH], f32, name="fr")
    nc.vector.tensor_copy(fr[0:1, :], pbig[0:1, 2048:2048 + NH])
    nc.scalar.activation(fr[32:33, :], pbig[32:33, 2048:2048 + NH], AF.Copy)

    # Whaug = x @ [W | wsrc | wdst] -> per i-tile [128, D+2]
    whl = []
    fsd = pers.tile([128, 2 * NT], f32, name="fsd")
    for it in range(NT):
        pwh = pbig[0:128, (it % 4) * 512:(it % 4) * 512 + D + 2]
        for k in range(2):
            nc.tensor.matmul(pwh, lhsT=xt[k][:, it * 128:(it + 1) * 128],
                             rhs=wf[k][:], start=(k == 0), stop=(k == 1))
        t = pers.tile([128, D + 1], bf, name=f"whl{it}")
        nc.vector.tensor_copy(t[:, 0:D], pwh[:, 0:D])
        nc.gpsimd.memset(t[:, D:D + 1], 1.0)
        nc.vector.tensor_copy(fsd[:, 2 * it:2 * it + 2], pwh[:, D:D + 2])
        whl.append(t)

    # broadcast fsrc to all partitions; A8b = exp(0.8*fs_i).
    # NB: partition_broadcast on HW only reads from partition 0, so the
    # offset source row is first DMA-shifted to partition 0.
    frb = xtpool.tile([33, NH], bf, name="frb")
    nc.vector.tensor_copy(frb[0:1, :], fr[0:1, :])
    nc.vector.tensor_copy(frb[32:33, :], fr[32:33, :])
    frb2 = xtpool.tile([1, NH], bf, name="frb2")
    nc.sync.dma_start(frb2[0:1, :], frb[32:33, :])
    fsb = pers.tile([128, n], bf, name="fsb")
    nc.gpsimd.partition_broadcast(fsb[:, 0:NH], frb[0:1, :])
    nc.gpsimd.partition_broadcast(fsb[:, NH:n], frb2[0:1, :])
    A8b = pers.tile([128, n], bf, name="A8b")
    nc.scalar.activation(A8b[:], fsb[:], AF.Exp, scale=0.8)
    xtp.__exit__(None, None, None)

    # per-partition fd constants: raw fd, 0.2*fd, exp(fd), exp(0.2*fd)
    fsdr = fsd.rearrange("p (t two) -> p t two", two=2)
    fdc = pers.tile([128, NT], f32, name="fdc")
    fd02 = pers.tile([128, NT], f32, name="fd02")
    Bc = pers.tile([128, NT], f32, name="Bc")
    Dc = pers.tile([128, NT], f32, name="Dc")
    fdcr = fdc.rearrange("p (t o) -> p t o", o=1)
    fd02r = fd02.rearrange("p (t o) -> p t o", o=1)
    Bcr = Bc.rearrange("p (t o) -> p t o", o=1)
    Dcr = Dc.rearrange("p (t o) -> p t o", o=1)
    nc.vector.tensor_copy(fdcr[:], fsdr[:, :, 1:2])
    nc.vector.tensor_scalar(out=fd02r[:], in0=fsdr[:, :, 1:2], scalar1=SLOPE,
                            scalar2=None, op0=OP.mult)
    nc.scalar.activation(Bcr[:], fsdr[:, :, 1:2], AF.Exp)
    nc.scalar.activation(Dcr[:], fsdr[:, :, 1:2], AF.Exp, scale=SLOPE)

    # woaug = [Wo_h | Wo_h@ao_src | Wo_h@ao_dst]  [D, C+2], duplicated on
    # partitions 0:64 and 64:128 (matmul requires lhsT/rhs base partitions
    # to match; eluO halves live at 0 and 64)
    woTt = pers.tile([128, D], bf, name="woTt")
    nc.gpsimd.memset(woTt[:], 0.0)
    nc.sync.dma_start(woTt[0:C, :], woT_d[:])
    ao2t = pers.tile([128, 2], bf, name="ao2t")
    nc.gpsimd.memset(ao2t[:], 0.0)
    nc.sync.dma_start(ao2t[0:C, :], ao2_d[:])
    woaug = pers.tile([128, C + 2], bf, name="woaug")
    for half in range(2):
        pwo = pbig[half * 64:half * 64 + D, 0:2]
        nc.tensor.matmul(pwo, lhsT=woTt[:, 0:D], rhs=ao2t[:],
                         start=True, stop=True)
        nc.sync.dma_start(woaug[half * 64:half * 64 + D, 0:C], wo_d[:])
        nc.vector.tensor_copy(woaug[half * 64:half * 64 + D, C:C + 2], pwo)

    I128 = pers.tile([128, 128], f32, name="I128")
    make_identity(nc, I128[:])

    # ---- layer-1 j-loop ----
    l1es = ExitStack()
    adj_pool = l1es.enter_context(tc.tile_pool(name="adj", bufs=BUFS))
    t_pool = l1es.enter_context(tc.tile_pool(name="t1", bufs=BUFS))
    u_pool = l1es.enter_context(tc.tile_pool(name="t2", bufs=BUFS))
    p_pool = l1es.enter_context(tc.tile_pool(name="pp", bufs=BUFS))

    act_set = _spread(ACT1_N, NT)
    # masks to gpsimd: prefer ACT-form tiles (their chains avoid DVE)
    order = [t for t in range(NT) if t in act_set] + \
            [t for t in range(NT) if t not in act_set]
    gps_set = set(order[:GPS1_N])

    # layer-2 adj slice, prefetched mid-loop (DMA has slack there) so the
    # transfer is done before the collective window
    adjs2 = pers.tile([128, NT * shard], bf, name="adjs2")
    adjs2v = adjs2.rearrange("p (t i) -> p t i", t=NT)

    nchunk = n // 512
    for t in range(NT):
        adjt = adj_pool.tile([128, n], bf, tag="adjt")
        eng = nc.sync if t % 2 == 0 else nc.scalar
        eng.dma_start(adjt[:], adjT_d[t * 128:(t + 1) * 128, :])
        if t == 20:
            nc.scalar.dma_start(
                adjs2v[:], adjs_d.ap().rearrange("(t p) i -> p t i", p=128))
        P = p_pool.tile([128, n], bf, tag="P")
        if t in act_set:
            tt1 = t_pool.tile([128, n], bf, tag="tt1")
            nc.scalar.activation(tt1[:], fsb[:], AF.Relu,
                                 bias=fdc[:, t:t + 1])
            uu = u_pool.tile([128, n], bf, tag="uu")
            nc.scalar.activation(uu[:], tt1[:], AF.Exp, scale=0.8,
                                 bias=fd02[:, t:t + 1])
        else:
            tt1 = t_pool.tile([128, n], bf, tag="tt1")
            nc.vector.tensor_scalar(out=tt1[:], in0=A8b[:],
                                    scalar1=Bc[:, t:t + 1],
                                    scalar2=None, op0=OP.mult)
            uu = u_pool.tile([128, n], bf, tag="uu")
            nc.vector.tensor_scalar(out=uu[:], in0=tt1[:],
                                    scalar1=Dc[:, t:t + 1],
                                    scalar2=None, op0=OP.max)
        if t in gps_set:
            nc.gpsimd.tensor_tensor(P[:], uu[:], adjt[:], OP.mult)
        else:
            nc.vector.tensor_tensor(P[:], uu[:], adjt[:], OP.mult)
        for c in range(nchunk):
            nc.tensor.matmul(pbig[0:D + 1, c * 512:(c + 1) * 512],
                             lhsT=whl[t][:], rhs=P[:, c * 512:(c + 1) * 512],
                             start=(t == 0), stop=(t == NT - 1))
    l1es.close()

    # ---- layer-1 epilogue: normalize + elu (split [128, NH] layout) ----
    # psum -> sbuf (partition-aligned compute copies on two engines);
    # transient tiles live in a scoped pool freed before layer-2 prep
    epp = tc.tile_pool(name="epp", bufs=1)
    ep = epp.__enter__()
    o1lo = ep.tile([D + 1, NH], f32, name="o1lo")
    o1hi = ep.tile([D + 1, NH], f32, name="o1hi")
    nc.vector.tensor_copy(o1lo[:], pbig[0:D + 1, 0:NH])
    nc.scalar.activation(o1hi[:], pbig[0:D + 1, NH:n], AF.Copy)
    # sbuf->sbuf DMAs to fold into a [128, NH] split layout
    o1s = ep.tile([128, NH], f32, name="o1s")
    nc.sync.dma_start(o1s[0:D, :], o1lo[0:D, :])
    nc.sync.dma_start(o1s[D:2 * D, :], o1hi[0:D, :])
    # 1/Z via exp(-ln(Z)) on the scalar engine (DVE reciprocal has a
    # ~5.3us fixed cost): shift Z rows to partition 0, invert, broadcast.
    # Lns then Exps batched to avoid activation-table swaps.
    zfa = ep.tile([1, NH], f32, name="zfa")
    zfb = ep.tile([1, NH], f32, name="zfb")
    zla = ep.tile([1, NH], bf, name="zla")
    zlb = ep.tile([1, NH], bf, name="zlb")
    zb = ep.tile([128, NH], bf, name="zb")
    zbx = ep.tile([D, NH], bf, name="zbx")
    nc.sync.dma_start(zfa[0:1, :], o1lo[D:D + 1, :])
    nc.sync.dma_start(zfb[0:1, :], o1hi[D:D + 1, :])
    nc.scalar.activation(zfa[0:1, :], zfa[0:1, :], AF.Ln)
    nc.scalar.activation(zfb[0:1, :], zfb[0:1, :], AF.Ln)
    nc.scalar.activation(zla[0:1, :], zfa[0:1, :], AF.Exp, scale=-1.0)
    nc.scalar.activation(zlb[0:1, :], zfb[0:1, :], AF.Exp, scale=-1.0)
    nc.gpsimd.partition_broadcast(zb[0:D, :], zla[0:1, :])
    nc.gpsimd.partition_broadcast(zbx[0:D, :], zlb[0:1, :])
    nc.sync.dma_start(zb[D:2 * D, :], zbx[0:D, :])
    o1n = ep.tile([128, NH], bf, name="o1n")
    nc.vector.tensor_tensor(o1n[:], o1s[:], zb[:], OP.mult)
    # elu
    mm = ep.tile([128, NH], bf, name="mm")
    nc.vector.tensor_scalar(out=mm[:], in0=o1n[:], scalar1=0.0, scalar2=None,
                            op0=OP.min)
    em = ep.tile([128, NH], bf, name="em")
    nc.scalar.activation(em[:], mm[:], AF.Exp)
    r1 = ep.tile([128, NH], bf, name="r1")
    nc.vector.tensor_scalar(out=r1[:], in0=o1n[:], scalar1=0.0, scalar2=-1.0,
                            op0=OP.max, op1=OP.add)
    eluO = pers.tile([128, NH], bf, name="eluO")
    nc.vector.tensor_tensor(eluO[:], r1[:], em[:], OP.add)

    # partial Who = eluO^T.T @ wo -> [n, C] into gts (col C holds 1/8 so the
    # AllReduce sum yields the ones column used for Z2); DMA to who region
    gts = pers.tile([128, NT, C1], bf, name="gts")
    nc.gpsimd.memset(gts[:, :, C:C + 1], 1.0 / CORES)
    half_t = NT // 2
    for it in range(NT):
        prt = (it // half_t) * D
        col = (it % half_t) * 128
        pt2 = pbig[0:128, (it % 8) * 512:(it % 8) * 512 + C]
        nc.tensor.matmul(pt2, lhsT=eluO[prt:prt + D, col:col + 128],
                         rhs=woaug[prt:prt + D, 0:C], start=True, stop=True)
        if it % 2 == 0:
            nc.vector.tensor_copy(gts[:, it, 0:C], pt2)
        else:
            nc.scalar.activation(gts[:, it, 0:C], pt2, AF.Copy)
        if it % 4 == 3:  # grouped stores: 8 DMA issues instead of 32
            g4 = it // 4
            nc.sync.dma_start(
                who_w[g4 * 512:(g4 + 1) * 512, :].rearrange(
                    "(t p) c -> p t c", p=128),
                gts[:, g4 * 4:g4 * 4 + 4, :])

    # gT = [g_src | g_dst]^T as [2, n]: out[r, i] = sum_d ao2[d, r]*eluO[d, i]
    for cc in range(n_cc):
        half = cc // cpg
        col = (cc % cpg) * 512
        pg = pbig[0:2, cc * 512:(cc + 1) * 512]
        nc.tensor.matmul(pg, lhsT=woaug[half * 64:half * 64 + D, C:C + 2],
                         rhs=eluO[half * 64:half * 64 + D, col:col + 512],
                         start=True, stop=True)
    gtt = ep.tile([2, n], bf, name="gtt")
    nc.vector.tensor_copy(gtt[:, 0:NH], pbig[0:2, 0:NH])
    nc.scalar.activation(gtt[:, NH:n], pbig[0:2, NH:n], AF.Copy)
    nc.sync.dma_start(g_w[:], gtt[:])
    epp.__exit__(None, None, None)

    # ---- collective: single AllReduce over the flat payload ----
    nc.gpsimd.collective_compute(
        "AllReduce", mybir.AluOpType.add, replica_groups=RG,
        ins=[rs_in.ap().rearrange("one (a b) -> (one a) b", a=CORES)],
        outs=[ag_out.ap().rearrange("one (a b) -> (one a) b", a=CORES)])

    # ---- layer-2 prep ----
    whol = pers.tile([128, NT, C1], bf, name="whol")
    nc.sync.dma_start(whol[:],
                      who_r.rearrange("(t p) c -> p t c", p=128))
    # g_dst -> [128, NT] via partition-split DMA of the flat row
    gdc = pers.tile([128, NT], bf, name="gdc")
    nc.sync.dma_start(
        gdc[:], g_r[1:2, :].rearrange("one (t p) -> (one p) t", p=128))
    gdf = pers.tile([128, NT], f32, name="gdf")
    gd02 = pers.tile([128, NT], f32, name="gd02")
    B2c = pers.tile([128, NT], f32, name="B2c")
    D2c = pers.tile([128, NT], f32, name="D2c")
    nc.vector.tensor_copy(gdf[:], gdc[:])
    nc.vector.tensor_scalar(out=gd02[:], in0=gdc[:], scalar1=SLOPE,
                            scalar2=None, op0=OP.mult)
    nc.scalar.activation(B2c[:], gdc[:], AF.Exp)
    nc.scalar.activation(D2c[:], gdc[:], AF.Exp, scale=SLOPE)
    # g_src slice for this core: one-hot rsel @ g_src viewed as [8, shard]
    gs8 = pers.tile([CORES, shard], bf, name="gs8")
    nc.sync.dma_start(
        gs8[:], g_r[0:1, :].rearrange("one (a i) -> (one a) i", a=CORES))
    pgs = pbig[0:1, 3584:3584 + shard]
    nc.tensor.matmul(pgs, lhsT=rselt[:], rhs=gs8[:], start=True, stop=True)
    gsr = pers.tile([1, shard], bf, name="gsr")
    nc.vector.tensor_copy(gsr[:], pgs)
    gsb = pers.tile([128, shard], bf, name="gsb")
    nc.gpsimd.partition_broadcast(gsb[:], gsr[0:1, :])
    A2b = pers.tile([128, shard], bf, name="A2b")
    nc.scalar.activation(A2b[:], gsb[:], AF.Exp, scale=0.8)

    # ---- layer-2 j-loop ----
    t_pool = es.enter_context(tc.tile_pool(name="t1b", bufs=BUFS))
    u_pool = es.enter_context(tc.tile_pool(name="t2b", bufs=BUFS))
    p_pool = es.enter_context(tc.tile_pool(name="ppb", bufs=BUFS))
    act2_set = _spread(ACT2_N, NT)
    order2 = [t for t in range(NT) if t in act2_set] + \
             [t for t in range(NT) if t not in act2_set]
    gps2_set = set(order2[:GPS2_N])
    for t in range(NT):
        adjs = adjs2[:, t * shard:(t + 1) * shard]
        P2 = p_pool.tile([128, shard], bf, tag="P2")
        if t in act2_set:
            q1 = t_pool.tile([128, shard], bf, tag="q1")
            nc.scalar.activation(q1[:], gsb[:], AF.Relu,
                                 bias=gdf[:, t:t + 1])
            q2 = u_pool.tile([128, shard], bf, tag="q2")
            nc.scalar.activation(q2[:], q1[:], AF.Exp, scale=0.8,
                                 bias=gd02[:, t:t + 1])
        else:
            q1 = t_pool.tile([128, shard], bf, tag="q1")
            nc.vector.tensor_scalar(out=q1[:], in0=A2b[:],
                                    scalar1=B2c[:, t:t + 1],
                                    scalar2=None, op0=OP.mult)
            q2 = u_pool.tile([128, shard], bf, tag="q2")
            nc.vector.tensor_scalar(out=q2[:], in0=q1[:],
                                    scalar1=D2c[:, t:t + 1],
                                    scalar2=None, op0=OP.max)
        if t in gps2_set:
            nc.gpsimd.tensor_tensor(P2[:], q2[:], adjs, OP.mult)
        else:
            nc.vector.tensor_tensor(P2[:], q2[:], adjs, OP.mult)
        nc.tensor.matmul(pbig[0:C + 1, 0:shard], lhsT=whol[:, t, :], rhs=P2[:],
                         start=(t == 0), stop=(t == NT - 1))

    if K_DEBUG:
        tap_fsd = nc.dram_tensor("tap_fsd", [128, 2 * NT], f32, kind="ExternalOutput")
        nc.sync.dma_start(tap_fsd.ap(), fsd[:])
        tap_o1lo = nc.dram_tensor("tap_o1lo", [D + 1, NH], f32, kind="ExternalOutput")
        nc.sync.dma_start(tap_o1lo.ap(), o1lo[:])
        tap_eluO = nc.dram_tensor("tap_eluO", [128, NH], bf, kind="ExternalOutput")
        nc.sync.dma_start(tap_eluO.ap(), eluO[:])
        tap_rsin = nc.dram_tensor("tap_rsin", [1, FLAT], bf, kind="ExternalOutput")
        nc.sync.dma_start(tap_rsin.ap(), rs_in.ap())
        tap_ag = nc.dram_tensor("tap_ag", [1, FLAT], bf, kind="ExternalOutput")
        nc.sync.dma_start(tap_ag.ap(), ag_out.ap())
        tap_gsb = nc.dram_tensor("tap_gsb", [128, shard], bf, kind="ExternalOutput")
        nc.sync.dma_start(tap_gsb.ap(), gsb[:])
        tap_gdf = nc.dram_tensor("tap_gdf", [128, NT], f32, kind="ExternalOutput")
        nc.sync.dma_start(tap_gdf.ap(), gdf[:])

    # ---- layer-2 epilogue: transpose, normalize, elu, log_softmax ----
    o2t = pers.tile([C + 1, shard], f32, name="o2t")
    nc.vector.tensor_copy(o2t[:], pbig[0:C + 1, 0:shard])
    if K_DEBUG:
        tap_o2t = nc.dram_tensor("tap_o2t", [C + 1, shard], f32, kind="ExternalOutput")
        nc.sync.dma_start(tap_o2t.ap(), o2t[:])
    # stage-major (all chunks per stage) so same-table ACT ops batch and
    # the engines pipeline across chunks
    nst = (shard + 127) // 128
    ws = [min(128, shard - k * 128) for k in range(nst)]
    ptrs = [pbig[0:ws[k], 512 + k * 512:512 + k * 512 + C + 1]
            for k in range(nst)]
    Tl = lambda nm, c=C: [pers.tile([128, c], f32, name=f"{nm}{k}")
                          for k in range(nst)]
    zr, o2n, m2, e2, r2, el2 = (Tl("zr", 1), Tl("o2n"), Tl("m2"), Tl("e2"),
                                Tl("r2"), Tl("el2"))
    mx, xm, ex, sume, lns, ok = (Tl("mx", 1), Tl("xm"), Tl("ex"),
                                 Tl("sume", 1), Tl("lns", 1), Tl("ok"))
    for k in range(nst):
        nc.tensor.transpose(ptrs[k], o2t[:, k * 128:k * 128 + ws[k]],
                            I128[0:C + 1, 0:C + 1])
    for k in range(nst):
        nc.scalar.activation(zr[k][0:ws[k], :], ptrs[k][:, C:C + 1], AF.Ln)
    for k in range(nst):
        nc.scalar.activation(zr[k][0:ws[k], :], zr[k][0:ws[k], :], AF.Exp,
                             scale=-1.0)
    for k in range(nst):
        w = ws[k]
        nc.vector.tensor_scalar(out=o2n[k][0:w, :], in0=ptrs[k][:, 0:C],
                                scalar1=zr[k][0:w, :], scalar2=None,
                                op0=OP.mult)
        nc.vector.tensor_scalar(out=m2[k][0:w, :], in0=o2n[k][0:w, :],
                                scalar1=0.0, scalar2=None, op0=OP.min)
    for k in range(nst):
        nc.scalar.activation(e2[k][0:ws[k], :], m2[k][0:ws[k], :], AF.Exp)
    for k in range(nst):
        w = ws[k]
        nc.vector.tensor_scalar(out=r2[k][0:w, :], in0=o2n[k][0:w, :],
                                scalar1=0.0, scalar2=-1.0, op0=OP.max,
                                op1=OP.add)
        nc.vector.tensor_tensor(el2[k][0:w, :], r2[k][0:w, :], e2[k][0:w, :],
                                OP.add)
        nc.vector.tensor_reduce(mx[k][0:w, :], el2[k][0:w, :],
                                mybir.AxisListType.X, OP.max)
        nc.vector.tensor_scalar(out=xm[k][0:w, :], in0=el2[k][0:w, :],
                                scalar1=mx[k][0:w, :], scalar2=None,
                                op0=OP.subtract)
    for k in range(nst):
        nc.scalar.activation(ex[k][0:ws[k], :], xm[k][0:ws[k], :], AF.Exp,
                             accum_out=sume[k][0:ws[k], :])
    for k in range(nst):
        nc.scalar.activation(lns[k][0:ws[k], :], sume[k][0:ws[k], :], AF.Ln)
    for k in range(nst):
        w = ws[k]
        nc.vector.tensor_scalar(out=ok[k][0:w, :], in0=xm[k][0:w, :],
                                scalar1=lns[k][0:w, :], scalar2=None,
                                op0=OP.subtract)
        nc.sync.dma_start(out_d[k * 128:k * 128 + w, :], ok[k][0:w, :])

    es.close()


def build(n=N, debug=False):
    from concourse import bacc
    import concourse.tile as tile

    nc = bacc.Bacc("TRN2", target_bir_lowering=False, debug=debug,
                   num_devices=CORES)
    with tile.TileContext(nc) as tc:
        _emit(nc, tc, n, n // CORES)
    nc.compile()
    return nc


def make_in_maps(x, adj, W, a, Wo, ao, n=N):
    """Host-side shard/layout prep -> list of 8 input dicts."""
    shard = n // CORES
    xT = np.ascontiguousarray(x.T).astype(BF)
    adjT = np.ascontiguousarray(adj.T).astype(BF)
    in_maps = []
    for h in range(CORES):
        wh = W[h].astype(BF)
        woh = Wo[h * D:(h + 1) * D, :].astype(BF)
        rsel = np.zeros((CORES, 1), dtype=BF)
        rsel[h, 0] = 1.0
        in_maps.append({
            "xT": xT,
            "adjT": adjT,
            "adjs": np.ascontiguousarray(adjT[:, h * shard:(h + 1) * shard]),
            "wh": wh,
            "whT": np.ascontiguousarray(wh.T),
            "a2": np.ascontiguousarray(np.stack([a[h, :D], a[h, D:]], axis=1)).astype(BF),
            "wo": woh,
            "woT": np.ascontiguousarray(woh.T),
            "ao2": np.ascontiguousarray(np.stack([ao[:C], ao[C:]], axis=1)).astype(BF),
            "rsel": rsel,
        })
    return in_maps


def kernel(x, adj, W, a, Wo, ao):
    from concourse.bass_utils import run_bass_kernel_spmd

    x = np.asarray(x, np.float32)
    adj = np.asarray(adj, np.float32)
    W = np.asarray(W, np.float32)
    a = np.asarray(a, np.float32)
    Wo = np.asarray(Wo, np.float32)
    ao = np.asarray(ao, np.float32)

    if "nc" not in _BASS_CACHE:
        _BASS_CACHE["nc"] = build()
    nc = _BASS_CACHE["nc"]
    in_maps = make_in_maps(x, adj, W, a, Wo, ao)
    r = run_bass_kernel_spmd(nc, in_maps, core_ids=list(range(CORES)))
    out = np.concatenate([r.results[c]["out"] for c in range(CORES)], axis=0)
    return np.asarray(out, np.float32)


# revision 23
# speedup vs baseline: 2.0590x; 1.1512x over previous
"""Trainium2 Bass kernel for a 2-layer GAT (nn_AGAEMD problem).

Sharding: layer-1 heads across 8 cores (core h owns head h, full N x N
attention for that head); layer-2 row-sharded (core c owns output rows
[c*512, (c+1)*512)).  Head outputs are combined with ONE bf16 AllReduce
over a flat contiguous payload (Who partials + a ones column + gT rows);
the per-core g_src slice is extracted post-AR with a one-hot selection
matmul (rsel input), avoiding any core-dependent addressing.

Math notes:
 - softmax rows are invariant to any per-column factor, so instead of
   P = exp(leaky(fs_i + fd_j))*adj we compute
   G2 = exp(0.8*relu(s) + 0.2*fd_j)*adj  (= P * exp(-0.2*fs_i)),
   which normalizes to the same attention.  Two equivalent pipelines:
     ACT-form: t1 = Relu(fsb + fd_j), t2 = Exp(0.8*t1 + 0.2*fd_j), mask
     DVE-form: u = A8b * B_j (ts), w = max(u, D_j) (ts), mask
   with A8b = exp(0.8*fs_i) broadcast, B = exp(fd), D = exp(0.2*fd).
 - reciprocals are computed as exp(-ln(x)) on the scalar engine (the
   DVE RECIPROCAL instruction costs ~5.3us regardless of size).
 - elu(x) = max(x,0) - 1 + exp(min(x,0)).
"""

import sys

if "/opt/trn_rl_repo" not in sys.path:
    sys.path.insert(0, "/opt/trn_rl_repo")

import numpy as np
import ml_dtypes

BF = ml_dtypes.bfloat16

# problem dims (hardcoded per spec)
N, F, H, D, C = 4096, 256, 8, 64, 64
CORES = 8
SLOPE = 0.2

import os as _os

# engine-split tunables: #ACT-form tiles (of 32) and #mask ops on gpsimd
ACT1_N = int(_os.environ.get("K_ACT1", "16"))
GPS1_N = int(_os.environ.get("K_GPS1", "0"))
ACT2_N = int(_os.environ.get("K_ACT2", "10"))
GPS2_N = int(_os.environ.get("K_GPS2", "0"))
BUFS = int(_os.environ.get("K_BUFS", "3"))
K_DEBUG = int(_os.environ.get("K_DEBUG", "0"))

_BASS_CACHE = {}


def _spread(k, nt):
    """k tile indices spread evenly over range(nt) (Bresenham)."""
    return {t for t in range(nt) if ((t + 1) * k) // nt > (t * k) // nt}


def _emit(nc, tc, n, shard):
    """Emit the SPMD per-core graph. n = graph size (4096 full), shard = n//8."""
    import concourse.bass as bass
    import concourse.mybir as mybir
    from concourse.masks import make_identity

    bf = mybir.dt.bfloat16
    f32 = mybir.dt.float32
    AF = mybir.ActivationFunctionType
    OP = mybir.AluOpType
    NT = n // 128          # number of 128-row tiles
    NH = n // 2            # split-layout free width
    RG = [list(range(CORES))]
    C1 = C + 1             # who payload row: C cols + ones col

    # ---- dram I/O ----
    xT_d = nc.dram_tensor("xT", [F, n], bf, kind="ExternalInput")
    adjT_d = nc.dram_tensor("adjT", [n, n], bf, kind="ExternalInput")
    adjs_d = nc.dram_tensor("adjs", [n, shard], bf, kind="ExternalInput")
    wh_d = nc.dram_tensor("wh", [F, D], bf, kind="ExternalInput")
    whT_d = nc.dram_tensor("whT", [D, F], bf, kind="ExternalInput")
    a2_d = nc.dram_tensor("a2", [D, 2], bf, kind="ExternalInput")
    wo_d = nc.dram_tensor("wo", [D, C], bf, kind="ExternalInput")
    woT_d = nc.dram_tensor("woT", [C, D], bf, kind="ExternalInput")
    ao2_d = nc.dram_tensor("ao2", [C, 2], bf, kind="ExternalInput")
    rsel_d = nc.dram_tensor("rsel", [CORES, 1], bf, kind="ExternalInput")
    out_d = nc.dram_tensor("out", [shard, C], f32, kind="ExternalOutput")

    # collective bounce buffers: flat payload = [n, C1] who rows + [2, n] gT
    FLAT = n * C1 + 2 * n
    rs_in = nc.dram_tensor("rs_in", [1, FLAT], bf)
    ag_out = nc.dram_tensor("ag_out", [1, FLAT], bf, addr_space="Shared")
    who_w = rs_in.ap()[0:1, 0:n * C1].rearrange("one (r c) -> (one r) c", c=C1)
    g_w = rs_in.ap()[0:1, n * C1:FLAT].rearrange("one (g i) -> (one g) i", i=n)
    who_r = ag_out.ap()[0:1, 0:n * C1].rearrange("one (r c) -> (one r) c", c=C1)
    g_r = ag_out.ap()[0:1, n * C1:FLAT].rearrange("one (g i) -> (one g) i", i=n)

    from contextlib import ExitStack

    es = ExitStack()
    pers = es.enter_context(tc.tile_pool(name="pers", bufs=1))
    ppool = es.enter_context(tc.tile_pool(name="psum", bufs=1, space="PSUM"))
    pbig = ppool.tile([128, 4096], f32, name="pbig")

    # ---- prologue: weights ----
    xtp = tc.tile_pool(name="xtp", bufs=1)
    xtpool = xtp.__enter__()
    xt = []
    for k in range(2):
        t = xtpool.tile([128, n], bf, name=f"xt{k}")
        nc.sync.dma_start(t[:], xT_d[k * 128:(k + 1) * 128, :])
        xt.append(t)
    wf = []
    for k in range(2):
        t = pers.tile([128, D + 2], bf, name=f"wf{k}")
        nc.sync.dma_start(t[:, 0:D], wh_d[k * 128:(k + 1) * 128, :])
        wf.append(t)
    whTt = pers.tile([128, F], bf, name="whTt")
    nc.gpsimd.memset(whTt[:], 0.0)
    nc.sync.dma_start(whTt[0:D, :], whT_d[:])
    a2t = pers.tile([128, 2], bf, name="a2t")
    nc.gpsimd.memset(a2t[:], 0.0)
    nc.sync.dma_start(a2t[0:D, :], a2_d[:])
    rselt = pers.tile([CORES, 1], bf, name="rselt")
    nc.sync.dma_start(rselt[:], rsel_d[:])

    # waug = W_h @ a2 : [F, 2] (two 128-row tiles)
    for k in range(2):
        pw = pbig[0:128, k * 512:k * 512 + 2]
        nc.tensor.matmul(pw, lhsT=whTt[:, k * 128:(k + 1) * 128], rhs=a2t[:],
                         start=True, stop=True)
        nc.vector.tensor_copy(wf[k][:, D:D + 2], pw)

    # fsrc row via matmul: fsrc = x @ wsrc -> psum rows, split on partitions 0/32
    n_cc = n // 512
    cpg = max(1, NH // 512)  # 512-chunks per half
    for cc in range(n_cc):
        part = (cc // cpg) * 32  # matmul out base partition must be 0/32/64
        foff = 2048 + 512 * (cc % cpg)
        pr = pbig[part:part + 1, foff:foff + 512]
        for k in range(2):
            nc.tensor.matmul(pr, lhsT=wf[k][:, D:D + 1],
                             rhs=xt[k][:, cc * 512:(cc + 1) * 512],
                             start=(k == 0), stop=(k == 1))
    # copy psum fsrc rows -> sbuf (partition-aligned; rows 0 and 32)
    fr = xtpool.tile([33, NH], f32, name="fr")
    nc.vector.tensor_copy(fr[0:1, :], pbig[0:1, 2048:2048 + NH])
    nc.scalar.activation(fr[32:33, :], pbig[32:33, 2048:2048 + NH], AF.Copy)

    # Whaug = x @ [W | wsrc | wdst] -> per i-tile [128, D+2]
    whl = []
    fsd = pers.tile([128, 2 * NT], f32, name="fsd")
    for it in range(NT):
        pwh = pbig[0:128, (it % 4) * 512:(it % 4) * 512 + D + 2]
        for k in range(2):
            nc.tensor.matmul(pwh, lhsT=xt[k][:, it * 128:(it + 1) * 128],
                             rhs=wf[k][:], start=(k == 0), stop=(k == 1))
        t = pers.tile([128, D + 1], bf, name=f"whl{it}")
        nc.vector.tensor_copy(t[:, 0:D], pwh[:, 0:D])
        nc.gpsimd.memset(t[:, D:D + 1], 1.0)
        nc.vector.tensor_copy(fsd[:, 2 * it:2 * it + 2], pwh[:, D:D + 2])
        whl.append(t)

    # broadcast fsrc to all partitions; A8b = exp(0.8*fs_i).
    # NB: partition_broadcast on HW only reads from partition 0, so the
    # offset source row is first DMA-shifted to partition 0.
    frb = xtpool.tile([33, NH], bf, name="frb")
    nc.vector.tensor_copy(frb[0:1, :], fr[0:1, :])
    nc.vector.tensor_copy(frb[32:33, :], fr[32:33, :])
    frb2 = xtpool.tile([1, NH], bf, name="frb2")
    nc.sync.dma_start(frb2[0:1, :], frb[32:33, :])
    fsb = pers.tile([128, n], bf, name="fsb")
    nc.gpsimd.partition_broadcast(fsb[:, 0:NH], frb[0:1, :])
    nc.gpsimd.partition_broadcast(fsb[:, NH:n], frb2[0:1, :])
    A8b = pers.tile([128, n], bf, name="A8b")
    nc.scalar.activation(A8b[:], fsb[:], AF.Exp, scale=0.8)
    xtp.__exit__(None, None, None)

    # per-partition fd constants: raw fd, 0.2*fd, exp(fd), exp(0.2*fd)
    fsdr = fsd.rearrange("p (t two) -> p t two", two=2)
    fdc = pers.tile([128, NT], f32, name="fdc")
    fd02 = pers.tile([128, NT], f32, name="fd02")
    Bc = pers.tile([128, NT], f32, name="Bc")
    Dc = pers.tile([128, NT], f32, name="Dc")
    fdcr = fdc.rearrange("p (t o) -> p t o", o=1)
    fd02r = fd02.rearrange("p (t o) -> p t o", o=1)
    Bcr = Bc.rearrange("p (t o) -> p t o", o=1)
    Dcr = Dc.rearrange("p (t o) -> p t o", o=1)
    nc.vector.tensor_copy(fdcr[:], fsdr[:, :, 1:2])
    nc.vector.tensor_scalar(out=fd02r[:], in0=fsdr[:, :, 1:2], scalar1=SLOPE,
                            scalar2=None, op0=OP.mult)
    nc.scalar.activation(Bcr[:], fsdr[:, :, 1:2], AF.Exp)
    nc.scalar.activation(Dcr[:], fsdr[:, :, 1:2], AF.Exp, scale=SLOPE)

    # woaug = [Wo_h | Wo_h@ao_src | Wo_h@ao_dst]  [D, C+2], duplicated on
    # partitions 0:64 and 64:128 (matmul requires lhsT/rhs base partitions
    # to match; eluO halves live at 0 and 64)
    woTt = pers.tile([128, D], bf, name="woTt")
    nc.gpsimd.memset(woTt[:], 0.0)
    nc.sync.dma_start(woTt[0:C, :], woT_d[:])
    ao2t = pers.tile([128, 2], bf, name="ao2t")
    nc.gpsimd.memset(ao2t[:], 0.0)
    nc.sync.dma_start(ao2t[0:C, :], ao2_d[:])
    woaug = pers.tile([128, C + 2], bf, name="woaug")
    for half in range(2):
        pwo = pbig[half * 64:half * 64 + D, 0:2]
        nc.tensor.matmul(pwo, lhsT=woTt[:, 0:D], rhs=ao2t[:],
                         start=True, stop=True)
        nc.sync.dma_start(woaug[half * 64:half * 64 + D, 0:C], wo_d[:])
        nc.vector.tensor_copy(woaug[half * 64:half * 64 + D, C:C + 2], pwo)

    I128 = pers.tile([128, 128], f32, name="I128")
    make_identity(nc, I128[:])

    # ---- layer-1 j-loop ----
    l1es = ExitStack()
    adj_pool = l1es.enter_context(tc.tile_pool(name="adj", bufs=BUFS))
    t_pool = l1es.enter_context(tc.tile_pool(name="t1", bufs=BUFS))
    u_pool = l1es.enter_context(tc.tile_pool(name="t2", bufs=BUFS))
    p_pool = l1es.enter_context(tc.tile_pool(name="pp", bufs=BUFS))

    act_set = _spread(ACT1_N, NT)
    # masks to gpsimd: prefer ACT-form tiles (their chains avoid DVE)
    order = [t for t in range(NT) if t in act_set] + \
            [t for t in range(NT) if t not in act_set]
    gps_set = set(order[:GPS1_N])

    # layer-2 adj slice, prefetched mid-loop (DMA has slack there) so the
    # transfer is done before the collective window
    adjs2 = pers.tile([128, NT * shard], bf, name="adjs2")
    adjs2v = adjs2.rearrange("p (t i) -> p t i", t=NT)

    nchunk = n // 512
    for t in range(NT):
        adjt = adj_pool.tile([128, n], bf, tag="adjt")
        eng = nc.sync if t % 2 == 0 else nc.scalar
        eng.dma_start(adjt[:], adjT_d[t * 128:(t + 1) * 128, :])
        if t == 20:
            nc.scalar.dma_start(
                adjs2v[:], adjs_d.ap().rearrange("(t p) i -> p t i", p=128))
        P = p_pool.tile([128, n], bf, tag="P")
        if t in act_set:
            tt1 = t_pool.tile([128, n], bf, tag="tt1")
            nc.scalar.activation(tt1[:], fsb[:], AF.Relu,
                                 bias=fdc[:, t:t + 1])
            uu = u_pool.tile([128, n], bf, tag="uu")
            nc.scalar.activation(uu[:], tt1[:], AF.Exp, scale=0.8,
                                 bias=fd02[:, t:t + 1])
        else:
            tt1 = t_pool.tile([128, n], bf, tag="tt1")
            nc.vector.tensor_scalar(out=tt1[:], in0=A8b[:],
                                    scalar1=Bc[:, t:t + 1],
                                    scalar2=None, op0=OP.mult)
            uu = u_pool.tile([128, n], bf, tag="uu")
            nc.vector.tensor_scalar(out=uu[:], in0=tt1[:],
                                    scalar1=Dc[:, t:t + 1],
                                    scalar2=None, op0=OP.max)
        if t in gps_set:
            nc.gpsimd.tensor_tensor(P[:], uu[:], adjt[:], OP.mult)
        else:
            nc.vector.tensor_tensor(P[:], uu[:], adjt[:], OP.mult)
        for c in range(nchunk):
            nc.tensor.matmul(pbig[0:D + 1, c * 512:(c + 1) * 512],
                             lhsT=whl[t][:], rhs=P[:, c * 512:(c + 1) * 512],
                             start=(t == 0), stop=(t == NT - 1))
    l1es.close()

    # ---- layer-1 epilogue: normalize + elu (split [128, NH] layout) ----
    # psum -> sbuf (partition-aligned compute copies on two engines);
    # transient tiles live in a scoped pool freed before layer-2 prep
    epp = tc.tile_pool(name="epp", bufs=1)
    ep = epp.__enter__()
    o1lo = ep.tile([D + 1, NH], f32, name="o1lo")
    o1hi = ep.tile([D + 1, NH], f32, name="o1hi")
    nc.vector.tensor_copy(o1lo[:], pbig[0:D + 1, 0:NH])
    nc.scalar.activation(o1hi[:], pbig[0:D + 1, NH:n], AF.Copy)
    # sbuf->sbuf DMAs to fold into a [128, NH] split layout
    o1s = ep.tile([128, NH], f32, name="o1s")
    nc.sync.dma_start(o1s[0:D, :], o1lo[0:D, :])
    nc.sync.dma_start(o1s[D:2 * D, :], o1hi[0:D, :])
    # 1/Z via exp(-ln(Z)) on the scalar engine (DVE reciprocal has a
    # ~5.3us fixed cost): shift Z rows to partition 0, invert, broadcast.
    # Lns then Exps batched to avoid activation-table swaps.
    zfa = ep.tile([1, NH], f32, name="zfa")
    zfb = ep.tile([1, NH], f32, name="zfb")
    zla = ep.tile([1, NH], bf, name="zla")
    zlb = ep.tile([1, NH], bf, name="zlb")
    zb = ep.tile([128, NH], bf, name="zb")
    zbx = ep.tile([D, NH], bf, name="zbx")
    nc.sync.dma_start(zfa[0:1, :], o1lo[D:D + 1, :])
    nc.sync.dma_start(zfb[0:1, :], o1hi[D:D + 1, :])
    nc.scalar.activation(zfa[0:1, :], zfa[0:1, :], AF.Ln)
    nc.scalar.activation(zfb[0:1, :], zfb[0:1, :], AF.Ln)
    nc.scalar.activation(zla[0:1, :], zfa[0:1, :], AF.Exp, scale=-1.0)
    nc.scalar.activation(zlb[0:1, :], zfb[0:1, :], AF.Exp, scale=-1.0)
    nc.gpsimd.partition_broadcast(zb[0:D, :], zla[0:1, :])
    nc.gpsimd.partition_broadcast(zbx[0:D, :], zlb[0:1, :])
    nc.sync.dma_start(zb[D:2 * D, :], zbx[0:D, :])
    o1n = ep.tile([128, NH], bf, name="o1n")
    nc.vector.tensor_tensor(o1n[:], o1s[:], zb[:], OP.mult)
    # elu
    mm = ep.tile([128, NH], bf, name="mm")
    nc.vector.tensor_scalar(out=mm[:], in0=o1n[:], scalar1=0.0, scalar2=None,
                            op0=OP.min)
    em = ep.tile([128, NH], bf, name="em")
    nc.scalar.activation(em[:], mm[:], AF.Exp)
    r1 = ep.tile([128, NH], bf, name="r1")
    nc.vector.tensor_scalar(out=r1[:], in0=o1n[:], scalar1=0.0, scalar2=-1.0,
                            op0=OP.max, op1=OP.add)
    eluO = pers.tile([128, NH], bf, name="eluO")
    nc.vector.tensor_tensor(eluO[:], r1[:], em[:], OP.add)

    # partial Who = eluO^T.T @ wo -> [n, C] into gts (col C holds 1/8 so the
    # AllReduce sum yields the ones column used for Z2); DMA to who region
    gts = pers.tile([128, NT, C1], bf, name="gts")
    nc.gpsimd.memset(gts[:, :, C:C + 1], 1.0 / CORES)
    half_t = NT // 2
    for it in range(NT):
        prt = (it // half_t) * D
        col = (it % half_t) * 128
        pt2 = pbig[0:128, (it % 8) * 512:(it % 8) * 512 + C]
        nc.tensor.matmul(pt2, lhsT=eluO[prt:prt + D, col:col + 128],
                         rhs=woaug[prt:prt + D, 0:C], start=True, stop=True)
        if it % 2 == 0:
            nc.vector.tensor_copy(gts[:, it, 0:C], pt2)
        else:
            nc.scalar.activation(gts[:, it, 0:C], pt2, AF.Copy)
        if it % 4 == 3:  # grouped stores: 8 DMA issues instead of 32
            g4 = it // 4
            nc.sync.dma_start(
                who_w[g4 * 512:(g4 + 1) * 512, :].rearrange(
                    "(t p) c -> p t c", p=128),
                gts[:, g4 * 4:g4 * 4 + 4, :])

    # gT = [g_src | g_dst]^T as [2, n]: out[r, i] = sum_d ao2[d, r]*eluO[d, i]
    for cc in range(n_cc):
        half = cc // cpg
        col = (cc % cpg) * 512
        pg = pbig[0:2, cc * 512:(cc + 1) * 512]
        nc.tensor.matmul(pg, lhsT=woaug[half * 64:half * 64 + D, C:C + 2],
                         rhs=eluO[half * 64:half * 64 + D, col:col + 512],
                         start=True, stop=True)
    gtt = ep.tile([2, n], bf, name="gtt")
    nc.vector.tensor_copy(gtt[:, 0:NH], pbig[0:2, 0:NH])
    nc.scalar.activation(gtt[:, NH:n], pbig[0:2, NH:n], AF.Copy)
    nc.sync.dma_start(g_w[:], gtt[:])
    epp.__exit__(None, None, None)

    # ---- collective: single AllReduce over the flat payload ----
    nc.gpsimd.collective_compute(
        "AllReduce", mybir.AluOpType.add, replica_groups=RG,
        ins=[rs_in.ap().rearrange("one (a b) -> (one a) b", a=CORES)],
        outs=[ag_out.ap().rearrange("one (a b) -> (one a) b", a=CORES)])

    # ---- layer-2 prep ----
    whol = pers.tile([128, NT, C1], bf, name="whol")
    nc.sync.dma_start(whol[:],
                      who_r.rearrange("(t p) c -> p t c", p=128))
    # g_dst -> [128, NT] via partition-split DMA of the flat row
    gdc = pers.tile([128, NT], bf, name="gdc")
    nc.sync.dma_start(
        gdc[:], g_r[1:2, :].rearrange("one (t p) -> (one p) t", p=128))
    gdf = pers.tile([128, NT], f32, name="gdf")
    gd02 = pers.tile([128, NT], f32, name="gd02")
    B2c = pers.tile([128, NT], f32, name="B2c")
    D2c = pers.tile([128, NT], f32, name="D2c")
    nc.vector.tensor_copy(gdf[:], gdc[:])
    nc.vector.tensor_scalar(out=gd02[:], in0=gdc[:], scalar1=SLOPE,
                            scalar2=None, op0=OP.mult)
    nc.scalar.activation(B2c[:], gdc[:], AF.Exp)
    nc.scalar.activation(D2c[:], gdc[:], AF.Exp, scale=SLOPE)
    # g_src slice for this core: one-hot rsel @ g_src viewed as [8, shard]
    gs8 = pers.tile([CORES, shard], bf, name="gs8")
    nc.sync.dma_start(
        gs8[:], g_r[0:1, :].rearrange("one (a i) -> (one a) i", a=CORES))
    pgs = pbig[0:1, 3584:3584 + shard]
    nc.tensor.matmul(pgs, lhsT=rselt[:], rhs=gs8[:], start=True, stop=True)
    gsr = pers.tile([1, shard], bf, name="gsr")
    nc.vector.tensor_copy(gsr[:], pgs)
    gsb = pers.tile([128, shard], bf, name="gsb")
    nc.gpsimd.partition_broadcast(gsb[:], gsr[0:1, :])
    A2b = pers.tile([128, shard], bf, name="A2b")
    nc.scalar.activation(A2b[:], gsb[:], AF.Exp, scale=0.8)

    # ---- layer-2 j-loop ----
    t_pool = es.enter_context(tc.tile_pool(name="t1b", bufs=BUFS))
    u_pool = es.enter_context(tc.tile_pool(name="t2b", bufs=BUFS))
    p_pool = es.enter_context(tc.tile_pool(name="ppb", bufs=BUFS))
    act2_set = _spread(ACT2_N, NT)
    order2 = [t for t in range(NT) if t in act2_set] + \
             [t for t in range(NT) if t not in act2_set]
    gps2_set = set(order2[:GPS2_N])
    for t in range(NT):
        adjs = adjs2[:, t * shard:(t + 1) * shard]
        P2 = p_pool.tile([128, shard], bf, tag="P2")
        if t in act2_set:
            q1 = t_pool.tile([128, shard], bf, tag="q1")
            nc.scalar.activation(q1[:], gsb[:], AF.Relu,
                                 bias=gdf[:, t:t + 1])
            q2 = u_pool.tile([128, shard], bf, tag="q2")
            nc.scalar.activation(q2[:], q1[:], AF.Exp, scale=0.8,
                                 bias=gd02[:, t:t + 1])
        else:
            q1 = t_pool.tile([128, shard], bf, tag="q1")
            nc.vector.tensor_scalar(out=q1[:], in0=A2b[:],
                                    scalar1=B2c[:, t:t + 1],
                                    scalar2=None, op0=OP.mult)
            q2 = u_pool.tile([128, shard], bf, tag="q2")
            nc.vector.tensor_scalar(out=q2[:], in0=q1[:],
                                    scalar1=D2c[:, t:t + 1],
                                    scalar2=None, op0=OP.max)
        if t in gps2_set:
            nc.gpsimd.tensor_tensor(P2[:], q2[:], adjs, OP.mult)
        else:
            nc.vector.tensor_tensor(P2[:], q2[:], adjs, OP.mult)
        nc.tensor.matmul(pbig[0:C + 1, 0:shard], lhsT=whol[:, t, :], rhs=P2[:],
                         start=(t == 0), stop=(t == NT - 1))

    if K_DEBUG:
        tap_fsd = nc.dram_tensor("tap_fsd", [128, 2 * NT], f32, kind="ExternalOutput")
        nc.sync.dma_start(tap_fsd.ap(), fsd[:])
        tap_o1lo = nc.dram_tensor("tap_o1lo", [D + 1, NH], f32, kind="ExternalOutput")
        nc.sync.dma_start(tap_o1lo.ap(), o1lo[:])
        tap_eluO = nc.dram_tensor("tap_eluO", [128, NH], bf, kind="ExternalOutput")
        nc.sync.dma_start(tap_eluO.ap(), eluO[:])
        tap_rsin = nc.dram_tensor("tap_rsin", [1, FLAT], bf, kind="ExternalOutput")
        nc.sync.dma_start(tap_rsin.ap(), rs_in.ap())
        tap_ag = nc.dram_tensor("tap_ag", [1, FLAT], bf, kind="ExternalOutput")
        nc.sync.dma_start(tap_ag.ap(), ag_out.ap())
        tap_gsb = nc.dram_tensor("tap_gsb", [128, shard], bf, kind="ExternalOutput")
        nc.sync.dma_start(tap_gsb.ap(), gsb[:])
        tap_gdf = nc.dram_tensor("tap_gdf", [128, NT], f32, kind="ExternalOutput")
        nc.sync.dma_start(tap_gdf.ap(), gdf[:])

    # ---- layer-2 epilogue: transpose, normalize, elu, log_softmax ----
    o2t = pers.tile([C + 1, shard], f32, name="o2t")
    nc.vector.tensor_copy(o2t[:], pbig[0:C + 1, 0:shard])
    if K_DEBUG:
        tap_o2t = nc.dram_tensor("tap_o2t", [C + 1, shard], f32, kind="ExternalOutput")
        nc.sync.dma_start(tap_o2t.ap(), o2t[:])
    # stage-major (all chunks per stage) so same-table ACT ops batch and
    # the engines pipeline across chunks
    nst = (shard + 127) // 128
    ws = [min(128, shard - k * 128) for k in range(nst)]
    ptrs = [pbig[0:ws[k], 512 + k * 512:512 + k * 512 + C + 1]
            for k in range(nst)]
    Tl = lambda nm, c=C: [pers.tile([128, c], f32, name=f"{nm}{k}")
                          for k in range(nst)]
    zr, o2n, m2, e2, r2, el2 = (Tl("zr", 1), Tl("o2n"), Tl("m2"), Tl("e2"),
                                Tl("r2"), Tl("el2"))
    mx, xm, ex, sume, lns, ok = (Tl("mx", 1), Tl("xm"), Tl("ex"),
                                 Tl("sume", 1), Tl("lns", 1), Tl("ok"))
    for k in range(nst):
        nc.tensor.transpose(ptrs[k], o2t[:, k * 128:k * 128 + ws[k]],
                            I128[0:C + 1, 0:C + 1])
    for k in range(nst):
        nc.scalar.activation(zr[k][0:ws[k], :], ptrs[k][:, C:C + 1], AF.Ln)
    for k in range(nst):
        nc.scalar.activation(zr[k][0:ws[k], :], zr[k][0:ws[k], :], AF.Exp,
                             scale=-1.0)
    for k in range(nst):
        w = ws[k]
        nc.vector.tensor_scalar(out=o2n[k][0:w, :], in0=ptrs[k][:, 0:C],
                                scalar1=zr[k][0:w, :], scalar2=None,
                                op0=OP.mult)
        nc.vector.tensor_scalar(out=m2[k][0:w, :], in0=o2n[k][0:w, :],
                                scalar1=0.0, scalar2=None, op0=OP.min)
    for k in range(nst):
        nc.scalar.activation(e2[k][0:ws[k], :], m2[k][0:ws[k], :], AF.Exp)
    for k in range(nst):
        w = ws[k]
        nc.vector.tensor_scalar(out=r2[k][0:w, :], in0=o2n[k][0:w, :],
                                scalar1=0.0, scalar2=-1.0, op0=OP.max,
                                op1=OP.add)
        nc.vector.tensor_tensor(el2[k][0:w, :], r2[k][0:w, :], e2[k][0:w, :],
                                OP.add)
        nc.vector.tensor_reduce(mx[k][0:w, :], el2[k][0:w, :],
                                mybir.AxisListType.X, OP.max)
        nc.vector.tensor_scalar(out=xm[k][0:w, :], in0=el2[k][0:w, :],
                                scalar1=mx[k][0:w, :], scalar2=None,
                                op0=OP.subtract)
    for k in range(nst):
        nc.scalar.activation(ex[k][0:ws[k], :], xm[k][0:ws[k], :], AF.Exp,
                             accum_out=sume[k][0:ws[k], :])
    for k in range(nst):
        nc.scalar.activation(lns[k][0:ws[k], :], sume[k][0:ws[k], :], AF.Ln)
    for k in range(nst):
        w = ws[k]
        nc.vector.tensor_scalar(out=ok[k][0:w, :], in0=xm[k][0:w, :],
                                scalar1=lns[k][0:w, :], scalar2=None,
                                op0=OP.subtract)
        nc.sync.dma_start(out_d[k * 128:k * 128 + w, :], ok[k][0:w, :])

    es.close()


def build(n=N, debug=False):
    from concourse import bacc
    import concourse.tile as tile

    nc = bacc.Bacc("TRN2", target_bir_lowering=False, debug=debug,
                   num_devices=CORES)
    with tile.TileContext(nc) as tc:
        _emit(nc, tc, n, n // CORES)
    nc.compile()
    return nc


def make_in_maps(x, adj, W, a, Wo, ao, n=N):
    """Host-side shard/layout prep -> list of 8 input dicts."""
    shard = n // CORES
    xT = np.ascontiguousarray(x.T).astype(BF)
    adjT = np.ascontiguousarray(adj.T).astype(BF)
    in_maps = []
    for h in range(CORES):
        wh = W[h].astype(BF)
        woh = Wo[h * D:(h + 1) * D, :].astype(BF)
        rsel = np.zeros((CORES, 1), dtype=BF)
        rsel[h, 0] = 1.0
        in_maps.append({
            "xT": xT,
            "adjT": adjT,
            "adjs": np.ascontiguousarray(adjT[:, h * shard:(h + 1) * shard]),
            "wh": wh,
            "whT": np.ascontiguousarray(wh.T),
            "a2": np.ascontiguousarray(np.stack([a[h, :D], a[h, D:]], axis=1)).astype(BF),
            "wo": woh,
            "woT": np.ascontiguousarray(woh.T),
            "ao2": np.ascontiguousarray(np.stack([ao[:C], ao[C:]], axis=1)).astype(BF),
            "rsel": rsel,
        })
    return in_maps


def kernel(x, adj, W, a, Wo, ao):
    from concourse.bass_utils import run_bass_kernel_spmd

    x = np.asarray(x, np.float32)
    adj = np.asarray(adj, np.float32)
    W = np.asarray(W, np.float32)
    a = np.asarray(a, np.float32)
    Wo = np.asarray(Wo, np.float32)
    ao = np.asarray(ao, np.float32)

    if "nc" not in _BASS_CACHE:
        _BASS_CACHE["nc"] = build()
    nc = _BASS_CACHE["nc"]
    in_maps = make_in_maps(x, adj, W, a, Wo, ao)
    r = run_bass_kernel_spmd(nc, in_maps, core_ids=list(range(CORES)))
    out = np.concatenate([r.results[c]["out"] for c in range(CORES)], axis=0)
    return np.asarray(out, np.float32)
